# revision 1
# baseline (speedup 1.0000x reference)
"""Multi-head self-attention (RoPE, causal) Bass kernel for 8 TRN2 NeuronCores.

Problem: x (2, 2048, 1024) f32, wqkv (3072, 1024), wo (1024, 1024).
  qkv = x @ wqkv.T ; RoPE(q, k) ; causal softmax attention (16 heads, hd=64);
  out = y @ wo.T.

Sharding: batch (2-way) x head-group (4-way) tensor parallel = 8 cores.
Each core computes a full (2048, 1024) partial output for its batch from its
4 heads; host sums the 4 partials per batch (the TP all-reduce done at
unshard time).

Precision: fp32r (hw-rounded fp32, ~1.5e-4) matmuls for QKV / scores;
fp16 (~2.4e-4) for P@V and the wo projection; exp and softmax sums in fp32.
"""
import sys

sys.path.insert(0, "/opt/trn_rl_repo")

import numpy as np

import concourse.bass as bass
import concourse.mybir as mybir
import concourse.tile as tile
from concourse import bacc, bass_utils
from concourse.masks import make_identity

B, L, D = 2, 2048, 1024
NH, HD = 16, 64
NCORES = 8
HPC = 4            # heads per core
LQB = 512          # Lq block per S^T unit
NLQ = L // LQB     # 4
NLT = L // 128     # 16
KT = D // 128      # 8 contraction tiles for projections

F32 = mybir.dt.float32
F32R = mybir.dt.float32r
F16 = mybir.dt.float16
BF16 = mybir.dt.bfloat16

_cache = {}


def build_nc(debug=False):
    nc = bacc.Bacc("TRN2", target_bir_lowering=False, debug=False)

    XT = nc.dram_tensor("XT", [D, L], F16, kind="ExternalInput")
    WQKT = nc.dram_tensor("WQKT", [D, 512], F16, kind="ExternalInput")
    WVT = nc.dram_tensor("WVT", [D, 260], F16, kind="ExternalInput")
    WOT = nc.dram_tensor("WOT", [HPC * HD, D], F16, kind="ExternalInput")
    PERM = nc.dram_tensor("PERM", [128, 128], F32R, kind="ExternalInput")
    COS = nc.dram_tensor("COS", [128, L], F32, kind="ExternalInput")
    SIN = nc.dram_tensor("SIN", [128, L], F32, kind="ExternalInput")
    NEGI = nc.dram_tensor("NEGI", [128, 128], BF16, kind="ExternalInput")
    BIGM = nc.dram_tensor("BIGM", [128, 128], BF16, kind="ExternalInput")
    OUT = nc.dram_tensor("OUT", [L, D], F32, kind="ExternalOutput")
    if debug:
        DROT = nc.dram_tensor("DROT", [512, L], F32, kind="ExternalOutput")
        DV = nc.dram_tensor("DV", [NLT * 128, 260], F16, kind="ExternalOutput")
        DY = nc.dram_tensor("DY", [NLT * 128, 256], F16, kind="ExternalOutput")
        DYT = nc.dram_tensor("DYT", [256, L], F16, kind="ExternalOutput")
        DSP = nc.dram_tensor("DSP", [128, 1024], F32, kind="ExternalOutput")
        DPT = nc.dram_tensor("DPT", [128, 1024], F16, kind="ExternalOutput")
        DYP = nc.dram_tensor("DYP", [128, 260], F32, kind="ExternalOutput")

    with tile.TileContext(nc) as tc:
        with (
            tc.tile_pool(name="consts", bufs=1) as cpool,
            tc.tile_pool(name="weights", bufs=1) as wpool,
            tc.tile_pool(name="qkrot", bufs=1) as rotpool,
            tc.tile_pool(name="vsb", bufs=1) as vpool,
            tc.tile_pool(name="yall", bufs=1) as ypool,
            tc.tile_pool(name="ytr", bufs=1) as ytpool,
            tc.tile_pool(name="xt", bufs=16) as xpool,
            tc.tile_pool(name="tmps", bufs=3) as tpool,
            tc.tile_pool(name="raws", bufs=3) as rawpool,
            tc.tile_pool(name="pts", bufs=4) as ptpool,
            tc.tile_pool(name="outsb", bufs=3) as opool,
            tc.tile_pool(name="recs", bufs=4) as recpool,
            tc.tile_pool(name="psB", bufs=3, space="PSUM") as bigpool,
            tc.tile_pool(name="psY", bufs=1, space="PSUM") as psypool,
        ):
            # ---- static loads, ordered by first use so the PE can start
            # as soon as wqk + the first x chunk land ---------------------
            def load_xt_tile(j, k):
                xs = slice(j * LQB, (j + 1) * LQB)
                t = xpool.tile([128, LQB], F16, tag="xt", name="xt")
                nc.sync.dma_start(t[:], XT[k * 128:(k + 1) * 128, xs])
                return t

            def load_xt(j):
                return [load_xt_tile(j, k) for k in range(KT)]

            # interleave wqk and xt0 k-tiles: the first qkv psum unit walks
            # k=0..7 in order, so it can start as soon as pair k=0 lands
            wqk_sb = []
            xt0 = []
            for k in range(KT):
                w = wpool.tile([128, 512], F16, tag=f"wqk{k}", name=f"wqk{k}")
                nc.sync.dma_start(w[:], WQKT[k * 128:(k + 1) * 128, :])
                wqk_sb.append(w)
                xt0.append(load_xt_tile(0, k))

            wvt_sb = []
            for k in range(KT):
                wv = wpool.tile([128, 260], F16, tag=f"wv{k}", name=f"wv{k}")
                nc.sync.dma_start(wv[:], WVT[k * 128:(k + 1) * 128, :])
                wvt_sb.append(wv)
            cos_sb = cpool.tile([128, L], F32, tag="cos")
            nc.sync.dma_start(cos_sb[:], COS[:, :])
            sin_sb = cpool.tile([128, L], F32, tag="sin")
            nc.sync.dma_start(sin_sb[:], SIN[:, :])
            perm_sb = cpool.tile([128, 128], F32R, tag="perm")
            nc.sync.dma_start(perm_sb[:], PERM[:, :])
            negi_sb = cpool.tile([128, 128], BF16, tag="negi")
            nc.sync.dma_start(negi_sb[:], NEGI[:, :])
            bigm_sb = cpool.tile([128, 128], BF16, tag="bigm")
            nc.sync.dma_start(bigm_sb[:], BIGM[:, :])
            idn = cpool.tile([128, 128], F16, tag="idn")
            make_identity(nc, idn[:])
            wot_sb = []
            for c2 in range(2):
                w = wpool.tile([128, D], F16, tag=f"wo{c2}", name=f"wo{c2}")
                nc.sync.dma_start(w[:], WOT[c2 * 128:(c2 + 1) * 128, :])
                wot_sb.append(w)

            # persistent activation storage
            # qk_rot[m]: m=0,1 -> q head-pairs (h01, h23); m=2,3 -> k pairs
            qk_rot = [rotpool.tile([128, L], F32R, tag=f"rot{m}", name=f"rot{m}")
                      for m in range(4)]
            v_sb = [vpool.tile([128, 260], F16, tag=f"v{t}", name=f"v{t}")
                    for t in range(NLT)]
            y_all = [ypool.tile([128, HPC * HD], F16, tag=f"y{i}", name=f"y{i}")
                     for i in range(NLT)]
            yt_sb = [ytpool.tile([128, L], F16, tag=f"yt{c2}", name=f"yt{c2}")
                     for c2 in range(2)]

            def qkv_chunk(j, xt):
                xs = slice(j * LQB, (j + 1) * LQB)
                # q/k head-pair tiles with rope
                for m in range(4):
                    big = bigpool.tile([128, 1024], F32, tag="big", name="big")
                    ps = big[:, 0:512]
                    psw = big[:, 512:1024]
                    for k in range(KT):
                        nc.tensor.matmul(
                            ps, wqk_sb[k][:, m * 128:(m + 1) * 128], xt[k][:],
                            start=(k == 0), stop=(k == KT - 1),
                        )
                    raw = rawpool.tile([128, LQB], F32R, tag="raw")
                    nc.scalar.copy(raw[:], ps)
                    t1 = tpool.tile([128, LQB], F32, tag="t1")
                    nc.vector.tensor_mul(t1[:], raw[:].bitcast(F32),
                                         cos_sb[:, xs])
                    nc.tensor.matmul(psw, perm_sb[:], raw[:],
                                     start=True, stop=True)
                    t2 = tpool.tile([128, LQB], F32, tag="t2")
                    nc.vector.tensor_mul(t2[:], psw, sin_sb[:, xs])
                    nc.vector.tensor_add(qk_rot[m][:, xs], t1[:], t2[:])
                # v tiles (natural L x hd layout, ones col after each head)
                for i2 in range(4):
                    ti = j * 4 + i2
                    bigv = bigpool.tile([128, 1024], F32, tag="big",
                                        name="bigv")
                    psv = bigv[:, 0:260]
                    for k in range(KT):
                        nc.tensor.matmul(
                            psv, xt[k][:, i2 * 128:(i2 + 1) * 128],
                            wvt_sb[k][:],
                            start=(k == 0), stop=(k == KT - 1),
                        )
                    nc.scalar.copy(v_sb[ti][:], psv)
                    nc.vector.memset(v_sb[ti][:, 64:260:65], 1.0)

            def attention_jq(jq):
                nt = 4 * jq + 4  # causal: Lk tiles 0 .. 4jq+3
                for hp in range(2):
                    y_ps = [
                        psypool.tile([128, 260], F32, tag=f"yps{h}",
                                     name=f"yps{h}", bufs=1)
                        for h in range(2)
                    ]
                    # zero via DVE, then accumulate with start=False
                    # throughout: matmul start=True zeroes the WHOLE psum
                    # bank, which clobbers sibling js-regions when Tile
                    # reorders the (commutative) accumulate matmuls.
                    for h in range(2):
                        nc.vector.memset(y_ps[h][:], 0.0)
                    for t in range(nt):
                        ks = slice(t * 128, (t + 1) * 128)
                        diag = t >= 4 * jq
                        # causal trim: cols < off are fully masked
                        off = max(0, t * 128 - jq * LQB)
                        r = off // 128
                        sp = bigpool.tile([128, 1024], F32, tag="big",
                                          name="sp")
                        for h in range(2):
                            hs = slice(64 * h, 64 * h + 64)
                            nc.tensor.matmul(
                                sp[:, 512 * h + off:512 * h + 512],
                                qk_rot[2 + hp][hs, ks],
                                qk_rot[hp][hs, jq * LQB + off:
                                           (jq + 1) * LQB],
                                start=True, stop=not diag,
                            )
                        if diag:
                            for h in range(2):
                                nc.tensor.matmul(
                                    sp[:, 512 * h + off:512 * h + off + 128],
                                    negi_sb[:],
                                    bigm_sb[:],
                                    start=False, stop=True,
                                )
                        pt = ptpool.tile([128, 1024], F16, tag="pt")
                        nc.scalar.activation(
                            pt[:, off:1024], sp[:, off:1024],
                            mybir.ActivationFunctionType.Exp
                        )
                        for h in range(2):
                            H = 2 * hp + h
                            for js in range(r, 4):
                                nc.tensor.matmul(
                                    y_ps[h][:, 65 * js:65 * js + 65],
                                    pt[:, 512 * h + 128 * js:
                                       512 * h + 128 * js + 128],
                                    v_sb[t][:, 65 * H:65 * H + 65],
                                    start=False, stop=(t == nt - 1),
                                    skip_group_check=True,
                                )
                    # normalize: y /= rowsum, write into y_all as fp16
                    for h in range(2):
                        H = 2 * hp + h
                        rec = recpool.tile([128, 4], F32, tag="rec")
                        nc.vector.reciprocal(rec[:], y_ps[h][:, 64:260:65])
                        for js in range(4):
                            i = 4 * jq + js
                            nc.vector.tensor_scalar_mul(
                                y_all[i][:, HD * H:HD * H + HD],
                                y_ps[h][:, 65 * js:65 * js + 64],
                                rec[:, js:js + 1],
                            )

            def wo_tiles(jq):
                for i in range(4 * jq, 4 * jq + 4):
                    bigt = bigpool.tile([128, 1024], F32, tag="big",
                                        name="bigt")
                    tp16 = bigt[:].bitcast(F16)   # (128, 2048) f16 view
                    for c2 in range(2):
                        nc.tensor.transpose(
                            tp16[:, 128 * c2:128 * c2 + 128],
                            y_all[i][:, 128 * c2:128 * c2 + 128],
                            idn[:],
                        )
                        nc.vector.tensor_copy(
                            yt_sb[c2][:, 128 * i:128 * i + 128],
                            tp16[:, 128 * c2:128 * c2 + 128],
                        )
                    po = bigpool.tile([128, 1024], F32, tag="big", name="po")
                    for half in range(2):
                        for c2 in range(2):
                            nc.tensor.matmul(
                                po[:, 512 * half:512 * half + 512],
                                yt_sb[c2][:, 128 * i:128 * i + 128],
                                wot_sb[c2][:, 512 * half:512 * half + 512],
                                start=(c2 == 0), stop=(c2 == 1),
                            )
                    ob = opool.tile([128, 1024], F32, tag="ob")
                    nc.any.tensor_copy(ob[:], po[:])
                    nc.gpsimd.dma_start(OUT[128 * i:128 * i + 128, :], ob[:])

            # defer each chunk's wo by one iteration: its inputs are long
            # ready, so the scheduler uses it to fill the PE bubble at the
            # attention(j) tail / qkv(j+1) seam (also keeps HAM warm there)
            for j in range(NLQ):
                xt = xt0 if j == 0 else load_xt(j)
                qkv_chunk(j, xt)
                if j > 0:
                    wo_tiles(j - 1)
                attention_jq(j)
            wo_tiles(NLQ - 1)

            if debug:
                for m in range(4):
                    nc.sync.dma_start(DROT[128 * m:128 * m + 128, :],
                                      qk_rot[m][:].bitcast(F32))
                for t in range(NLT):
                    nc.sync.dma_start(DV[128 * t:128 * t + 128, :], v_sb[t][:])
                for i in range(NLT):
                    nc.sync.dma_start(DY[128 * i:128 * i + 128, :], y_all[i][:])
                for c2 in range(2):
                    nc.sync.dma_start(DYT[128 * c2:128 * c2 + 128, :],
                                      yt_sb[c2][:])

    nc.finalize()
    return nc


def prep_inputs(x, wqkv, wo):
    """Build the 8 per-core input dicts from the full-problem inputs."""
    import ml_dtypes
    bf = ml_dtypes.bfloat16

    x = np.asarray(x, dtype=np.float32)
    wqkv = np.asarray(wqkv, dtype=np.float32)
    wo = np.asarray(wo, dtype=np.float32)

    # rope tables
    inv_freq = 1.0 / (10000.0 ** (np.arange(0, HD, 2, dtype=np.float32) / HD))
    t = np.arange(L, dtype=np.float32)
    freqs = np.outer(t, inv_freq)                  # (L, 32)
    cos32 = np.cos(freqs).T.astype(np.float32)     # (32, L)
    sin32 = np.sin(freqs).T.astype(np.float32)
    COS = np.ascontiguousarray(np.tile(cos32, (4, 1)))           # (128, L)
    SIN = np.ascontiguousarray(
        np.concatenate([-sin32, sin32, -sin32, sin32], axis=0)
    )

    # 32-block swap permutation (within each head's 64 rows)
    PERM = np.zeros((128, 128), dtype=np.float32)
    for blk in range(2):
        o = 64 * blk
        PERM[o:o + 32, o + 32:o + 64] = np.eye(32)
        PERM[o + 32:o + 64, o:o + 32] = np.eye(32)

    NEGI = (-1e9 * np.eye(128)).astype(bf)
    BIGM = (np.arange(128)[None, :] < np.arange(128)[:, None])
    BIGM = np.ascontiguousarray(BIGM.astype(np.float32)).astype(bf)

    in_maps = []
    scale = np.float32(HD ** -0.5)
    for c in range(NCORES):
        b, g = divmod(c, 4)
        qrows = slice(256 * g, 256 * g + 256)
        krows = slice(1024 + 256 * g, 1024 + 256 * g + 256)
        vrows = slice(2048 + 256 * g, 2048 + 256 * g + 256)

        XT = np.ascontiguousarray(x[b].T)                        # (1024, 2048)
        wq = (wqkv[qrows, :] * scale).T                          # (1024, 256)
        wk = wqkv[krows, :].T
        WQKT = np.ascontiguousarray(np.concatenate([wq, wk], axis=1))
        vpart = wqkv[vrows, :].T                                 # (1024, 256)
        WVT = np.zeros((D, 260), dtype=np.float32)
        for h in range(HPC):
            WVT[:, 65 * h:65 * h + 64] = vpart[:, 64 * h:64 * h + 64]
        WOT = np.ascontiguousarray(wo[:, 256 * g:256 * g + 256].T)

        in_maps.append({
            "XT": XT.astype(np.float16),
            "WQKT": WQKT.astype(np.float16),
            "WVT": WVT.astype(np.float16),
            "WOT": WOT.astype(np.float16),
            "COS": COS,
            "SIN": SIN,
            "PERM": PERM,
            "NEGI": NEGI,
            "BIGM": BIGM,
        })
    return in_maps


def kernel(x, wqkv, wo):
    if "nc" not in _cache:
        _cache["nc"] = build_nc()
    nc = _cache["nc"]
    in_maps = prep_inputs(x, wqkv, wo)
    res = bass_utils.run_bass_kernel_spmd(nc, in_maps, list(range(NCORES)))
    outs = [res.results[c]["OUT"] for c in range(NCORES)]
    out0 = outs[0] + outs[1] + outs[2] + outs[3]
    out1 = outs[4] + outs[5] + outs[6] + outs[7]
    return np.stack([out0, out1]).astype(np.float32)



# revision 4
# speedup vs baseline: 1.1207x; 1.1207x over previous
"""Multi-head self-attention (RoPE, causal) Bass kernel for 8 TRN2 NeuronCores.

Problem: x (2, 2048, 1024) f32, wqkv (3072, 1024), wo (1024, 1024).
  qkv = x @ wqkv.T ; RoPE(q, k) ; causal softmax attention (16 heads, hd=64);
  out = y @ wo.T.

Sharding: batch (2-way) x head-group (4-way) tensor parallel = 8 cores.
Each core computes a full (2048, 1024) partial output for its batch from its
4 heads; host sums the 4 partials per batch (the TP all-reduce done at
unshard time).

v2 structure (vs the fp32r baseline):
  - fp16 qk_rot / cos / sin / perm: the fp32r scores matmuls ran at half PE
    clock (fp32_mode=HIGH); fp16 runs 1 row/cycle.
  - causal diag masking via gpsimd affine_select on the exp'd P tile
    (zero upper-triangle) instead of -1e9 mask matmuls on the PE.
  - y_all -> yt transposes via SBUF->SBUF transpose DMA instead of PE
    transposes + DVE copies.
  - PSUM->SBUF copies (qkv raw, v) on gpsimd; the scalar engine runs only
    the softmax exp (the serial bottleneck of the attention phase).
  - software pipelining: qkv(j+1) and wo(j-1) matmul groups are emitted
    interleaved with attention(j) tile steps so the PE fills the exp-wait
    gaps; all input DMAs issued up front on SP in priority order.
"""
import sys

sys.path.insert(0, "/opt/trn_rl_repo")

import numpy as np

import concourse.bass as bass
import concourse.mybir as mybir
import concourse.tile as tile
from concourse import bacc, bass_utils

B, L, D = 2, 2048, 1024
NH, HD = 16, 64
NCORES = 8
HPC = 4            # heads per core
LQB = 512          # Lq block per S^T unit
NLQ = L // LQB     # 4
NLT = L // 128     # 16
KT = D // 128      # 8 contraction tiles for projections

F32 = mybir.dt.float32
F16 = mybir.dt.float16

_cache = {}


def build_nc(debug=False):
    nc = bacc.Bacc("TRN2", target_bir_lowering=False, debug=False)

    XT = nc.dram_tensor("XT", [D, L], F16, kind="ExternalInput")
    WQKT = nc.dram_tensor("WQKT", [D, 512], F16, kind="ExternalInput")
    WVT = nc.dram_tensor("WVT", [D, 260], F16, kind="ExternalInput")
    WOT = nc.dram_tensor("WOT", [HPC * HD, D], F16, kind="ExternalInput")
    PERM = nc.dram_tensor("PERM", [128, 128], F16, kind="ExternalInput")
    COS = nc.dram_tensor("COS", [128, L], F16, kind="ExternalInput")
    SIN = nc.dram_tensor("SIN", [128, L], F16, kind="ExternalInput")
    OUT = nc.dram_tensor("OUT", [L, D], F32, kind="ExternalOutput")

    with tile.TileContext(nc) as tc:
        with (
            tc.tile_pool(name="consts", bufs=1) as cpool,
            tc.tile_pool(name="weights", bufs=1) as wpool,
            tc.tile_pool(name="qkrot", bufs=1) as rotpool,
            tc.tile_pool(name="vsb", bufs=1) as vpool,
            tc.tile_pool(name="yall", bufs=1) as ypool,
            tc.tile_pool(name="ytr", bufs=1) as ytpool,
            tc.tile_pool(name="xt", bufs=1) as xpool,
            tc.tile_pool(name="tmps", bufs=4) as tpool,
            tc.tile_pool(name="raws", bufs=3) as rawpool,
            tc.tile_pool(name="pts", bufs=4) as ptpool,
            tc.tile_pool(name="outsb", bufs=3) as opool,
            tc.tile_pool(name="recs", bufs=4) as recpool,
            tc.tile_pool(name="psB", bufs=3, space="PSUM") as bigpool,
            tc.tile_pool(name="psY", bufs=1, space="PSUM") as psypool,
        ):
            # ---- all input DMAs up front on SP, priority-ordered ---------
            wqk_sb = [None] * KT
            xts = [[None] * KT for _ in range(NLQ)]
            cos_c = [None] * NLQ
            sin_c = [None] * NLQ

            def load_wqk(k):
                w = wpool.tile([128, 512], F16, tag=f"wqk{k}", name=f"wqk{k}")
                nc.sync.dma_start(w[:], WQKT[k * 128:(k + 1) * 128, :])
                wqk_sb[k] = w

            def load_xt(j, k):
                xs = slice(j * LQB, (j + 1) * LQB)
                t = xpool.tile([128, LQB], F16, tag=f"xt{j}_{k}",
                               name=f"xt{j}_{k}")
                nc.sync.dma_start(t[:], XT[k * 128:(k + 1) * 128, xs])
                xts[j][k] = t

            def load_cs(j):
                xs = slice(j * LQB, (j + 1) * LQB)
                c = cpool.tile([128, LQB], F16, tag=f"cos{j}")
                nc.sync.dma_start(c[:], COS[:, xs])
                cos_c[j] = c
                s = cpool.tile([128, LQB], F16, tag=f"sin{j}")
                nc.sync.dma_start(s[:], SIN[:, xs])
                sin_c[j] = s

            # chunk-0 critical path first: wqk+xt0 pairs, cos/sin chunk 0
            for k in range(KT):
                load_wqk(k)
                load_xt(0, k)
                if k == 1:
                    load_cs(0)
            wvt_sb = []
            for k in range(KT):
                wv = wpool.tile([128, 260], F16, tag=f"wv{k}", name=f"wv{k}")
                nc.sync.dma_start(wv[:], WVT[k * 128:(k + 1) * 128, :])
                wvt_sb.append(wv)
            for k in range(KT):
                load_xt(1, k)
            perm_sb = cpool.tile([128, 128], F16, tag="perm")
            nc.sync.dma_start(perm_sb[:], PERM[:, :])
            load_cs(1)
            wot_sb = []
            for c2 in range(2):
                w = wpool.tile([128, D], F16, tag=f"wo{c2}", name=f"wo{c2}")
                nc.sync.dma_start(w[:], WOT[c2 * 128:(c2 + 1) * 128, :])
                wot_sb.append(w)
            for k in range(KT):
                load_xt(2, k)
            load_cs(2)
            for k in range(KT):
                load_xt(3, k)
            load_cs(3)

            # persistent activation storage
            # qk_rot[m]: m=0,1 -> q head-pairs (h01, h23); m=2,3 -> k pairs
            qk_rot = [rotpool.tile([128, L], F16, tag=f"rot{m}",
                                   name=f"rot{m}")
                      for m in range(4)]
            v_sb = [vpool.tile([128, 260], F16, tag=f"v{t}", name=f"v{t}")
                    for t in range(NLT)]
            y_all = [ypool.tile([128, HPC * HD], F16, tag=f"y{i}", name=f"y{i}")
                     for i in range(NLT)]
            yt_sb = [ytpool.tile([128, L], F16, tag=f"yt{c2}", name=f"yt{c2}")
                     for c2 in range(2)]

            def qkv_m_group(j, m):
                xs = slice(j * LQB, (j + 1) * LQB)
                big = bigpool.tile([128, 1024], F32, tag="big", name="big")
                ps = big[:, 0:512]
                psw = big[:, 512:1024]
                for k in range(KT):
                    nc.tensor.matmul(
                        ps, wqk_sb[k][:, m * 128:(m + 1) * 128], xts[j][k][:],
                        start=(k == 0), stop=(k == KT - 1),
                    )
                raw = rawpool.tile([128, LQB], F16, tag="raw")
                nc.vector.tensor_copy(raw[:], ps)
                t1 = tpool.tile([128, LQB], F16, tag="t1")
                nc.vector.tensor_mul(t1[:], raw[:], cos_c[j][:])
                nc.tensor.matmul(psw, perm_sb[:], raw[:],
                                 start=True, stop=True)
                t2 = tpool.tile([128, LQB], F16, tag="t2")
                nc.vector.tensor_mul(t2[:], psw, sin_c[j][:])
                nc.vector.tensor_add(qk_rot[m][:, xs], t1[:], t2[:])

            def v_group(j, i2):
                ti = j * 4 + i2
                bigv = bigpool.tile([128, 1024], F32, tag="big", name="bigv")
                psv = bigv[:, 0:260]
                for k in range(KT):
                    nc.tensor.matmul(
                        psv, xts[j][k][:, i2 * 128:(i2 + 1) * 128],
                        wvt_sb[k][:],
                        start=(k == 0), stop=(k == KT - 1),
                    )
                nc.vector.tensor_copy(v_sb[ti][:], psv)
                nc.vector.memset(v_sb[ti][:, 64:260:65], 1.0)

            def qkv_chunk(j):
                for m in range(4):
                    qkv_m_group(j, m)
                for i2 in range(4):
                    v_group(j, i2)

            # ---- attention ----------------------------------------------
            y_ps = [None, None]

            def wave_start(jq, hp):
                for h in range(2):
                    y_ps[h] = psypool.tile([128, 260], F32, tag=f"yps{h}",
                                           name=f"yps{h}", bufs=1)
                    nc.vector.memset(y_ps[h][:], 0.0)

            def attn_step(jq, hp, t):
                nt = 4 * jq + 4
                ks = slice(t * 128, (t + 1) * 128)
                diag = t >= 4 * jq
                off = max(0, t * 128 - jq * LQB)
                r = off // 128
                sp = bigpool.tile([128, 1024], F32, tag="big", name="sp")
                for h in range(2):
                    hs = slice(64 * h, 64 * h + 64)
                    nc.tensor.matmul(
                        sp[:, 512 * h + off:512 * h + 512],
                        qk_rot[2 + hp][hs, ks],
                        qk_rot[hp][hs, jq * LQB + off:(jq + 1) * LQB],
                        start=True, stop=True,
                    )
                pt = ptpool.tile([128, 1024], F16, tag="pt")
                nc.scalar.activation(
                    pt[:, off:1024], sp[:, off:1024],
                    mybir.ActivationFunctionType.Exp
                )
                if diag:
                    # zero the upper triangle of the diagonal 128x128 block
                    # (post-exp): keep where q_local - k_local >= 0
                    for h in range(2):
                        blk = pt[:, 512 * h + off:512 * h + off + 128]
                        nc.gpsimd.affine_select(
                            out=blk, in_=blk,
                            pattern=[[1, 128]],
                            compare_op=mybir.AluOpType.is_ge,
                            fill=0.0, base=0, channel_multiplier=-1,
                        )
                for h in range(2):
                    H = 2 * hp + h
                    for js in range(r, 4):
                        nc.tensor.matmul(
                            y_ps[h][:, 65 * js:65 * js + 65],
                            pt[:, 512 * h + 128 * js:512 * h + 128 * js + 128],
                            v_sb[t][:, 65 * H:65 * H + 65],
                            start=False, stop=(t == nt - 1),
                            skip_group_check=True,
                        )

            def wave_end(jq, hp):
                # normalize: y /= rowsum, write into y_all as fp16
                for h in range(2):
                    H = 2 * hp + h
                    rec = recpool.tile([128, 4], F32, tag="rec")
                    nc.vector.reciprocal(rec[:], y_ps[h][:, 64:260:65])
                    for js in range(4):
                        i = 4 * jq + js
                        nc.vector.tensor_scalar_mul(
                            y_all[i][:, HD * H:HD * H + HD],
                            y_ps[h][:, 65 * js:65 * js + 64],
                            rec[:, js:js + 1],
                        )

            def transpose_tiles(jq):
                # y_all[i] (L-rows x d) -> yt (d x L) via SBUF->SBUF
                # transpose DMA (replaces PE transposes + DVE copies)
                for i in range(4 * jq, 4 * jq + 4):
                    for c2 in range(2):
                        nc.sync.dma_start(
                            yt_sb[c2][:, 128 * i:128 * i + 128],
                            y_all[i][:, 128 * c2:128 * c2 + 128],
                            transpose=True,
                        )

            def wo_tile(i):
                po = bigpool.tile([128, 1024], F32, tag="big", name="po")
                for half in range(2):
                    for c2 in range(2):
                        nc.tensor.matmul(
                            po[:, 512 * half:512 * half + 512],
                            yt_sb[c2][:, 128 * i:128 * i + 128],
                            wot_sb[c2][:, 512 * half:512 * half + 512],
                            start=(c2 == 0), stop=(c2 == 1),
                        )
                ob = opool.tile([128, 1024], F32, tag="ob")
                nc.any.tensor_copy(ob[:], po[:])
                nc.sync.dma_start(OUT[128 * i:128 * i + 128, :], ob[:])

            def attn_zip(jq, fillers):
                """Emit attention(jq) steps with filler work interleaved so
                the PE stays fed during the exp-gated stretches."""
                nt = 4 * jq + 4
                steps = []
                for hp in range(2):
                    steps.append(("ws", hp))
                    for t in range(nt):
                        steps.append(("st", hp, t))
                    steps.append(("we", hp))
                # only count "st" steps for filler spreading
                st_idx = [i for i, s in enumerate(steps) if s[0] == "st"]
                ns, nf = len(st_idx), len(fillers)
                fpos = {}
                for f in range(nf):
                    pos = st_idx[min(ns - 1, (f * ns) // nf)]
                    fpos.setdefault(pos, []).append(fillers[f])
                for i, s in enumerate(steps):
                    for fn in fpos.get(i, ()):
                        fn()
                    if s[0] == "ws":
                        wave_start(jq, s[1])
                    elif s[0] == "st":
                        attn_step(jq, s[1], s[2])
                    else:
                        wave_end(jq, s[1])

            # ---- main schedule ------------------------------------------
            qkv_chunk(0)
            for j in range(NLQ):
                fillers = []
                if j < NLQ - 1:
                    jn = j + 1
                    fillers += [
                        (lambda m=m, jn=jn: qkv_m_group(jn, m))
                        for m in range(4)
                    ]
                    fillers += [
                        (lambda i2=i2, jn=jn: v_group(jn, i2))
                        for i2 in range(4)
                    ]
                if j == 1:
                    fillers += [(lambda i=i: wo_tile(i)) for i in range(0, 4)]
                if j == 3:
                    fillers += [(lambda i=i: wo_tile(i)) for i in range(4, 12)]
                attn_zip(j, fillers)
                transpose_tiles(j)
            for i in range(12, 16):
                wo_tile(i)

    nc.finalize()
    return nc


def prep_inputs(x, wqkv, wo):
    """Build the 8 per-core input dicts from the full-problem inputs."""
    x = np.asarray(x, dtype=np.float32)
    wqkv = np.asarray(wqkv, dtype=np.float32)
    wo = np.asarray(wo, dtype=np.float32)

    # rope tables
    inv_freq = 1.0 / (10000.0 ** (np.arange(0, HD, 2, dtype=np.float32) / HD))
    t = np.arange(L, dtype=np.float32)
    freqs = np.outer(t, inv_freq)                  # (L, 32)
    cos32 = np.cos(freqs).T.astype(np.float32)     # (32, L)
    sin32 = np.sin(freqs).T.astype(np.float32)
    COS = np.ascontiguousarray(np.tile(cos32, (4, 1)))           # (128, L)
    SIN = np.ascontiguousarray(
        np.concatenate([-sin32, sin32, -sin32, sin32], axis=0)
    )

    # 32-block swap permutation (within each head's 64 rows)
    PERM = np.zeros((128, 128), dtype=np.float32)
    for blk in range(2):
        o = 64 * blk
        PERM[o:o + 32, o + 32:o + 64] = np.eye(32)
        PERM[o + 32:o + 64, o:o + 32] = np.eye(32)

    in_maps = []
    scale = np.float32(HD ** -0.5)
    for c in range(NCORES):
        b, g = divmod(c, 4)
        qrows = slice(256 * g, 256 * g + 256)
        krows = slice(1024 + 256 * g, 1024 + 256 * g + 256)
        vrows = slice(2048 + 256 * g, 2048 + 256 * g + 256)

        XT = np.ascontiguousarray(x[b].T)                        # (1024, 2048)
        wq = (wqkv[qrows, :] * scale).T                          # (1024, 256)
        wk = wqkv[krows, :].T
        WQKT = np.ascontiguousarray(np.concatenate([wq, wk], axis=1))
        vpart = wqkv[vrows, :].T                                 # (1024, 256)
        WVT = np.zeros((D, 260), dtype=np.float32)
        for h in range(HPC):
            WVT[:, 65 * h:65 * h + 64] = vpart[:, 64 * h:64 * h + 64]
        WOT = np.ascontiguousarray(wo[:, 256 * g:256 * g + 256].T)

        in_maps.append({
            "XT": XT.astype(np.float16),
            "WQKT": WQKT.astype(np.float16),
            "WVT": WVT.astype(np.float16),
            "WOT": WOT.astype(np.float16),
            "COS": COS.astype(np.float16),
            "SIN": SIN.astype(np.float16),
            "PERM": PERM.astype(np.float16),
        })
    return in_maps


def kernel(x, wqkv, wo):
    if "nc" not in _cache:
        _cache["nc"] = build_nc()
    nc = _cache["nc"]
    in_maps = prep_inputs(x, wqkv, wo)
    res = bass_utils.run_bass_kernel_spmd(nc, in_maps, list(range(NCORES)))
    outs = [res.results[c]["OUT"] for c in range(NCORES)]
    out0 = outs[0] + outs[1] + outs[2] + outs[3]
    out1 = outs[4] + outs[5] + outs[6] + outs[7]
    return np.stack([out0, out1]).astype(np.float32)


# revision 8
# speedup vs baseline: 1.1878x; 1.0599x over previous
"""Multi-head self-attention (RoPE, causal) Bass kernel for 8 TRN2 NeuronCores.

Problem: x (2, 2048, 1024) f32, wqkv (3072, 1024), wo (1024, 1024).
  qkv = x @ wqkv.T ; RoPE(q, k) ; causal softmax attention (16 heads, hd=64);
  out = y @ wo.T.

Sharding: batch (2-way) x head-group (4-way) tensor parallel = 8 cores.
Each core computes a full (2048, 1024) partial output for its batch from its
4 heads; host sums the 4 partials per batch (the TP all-reduce done at
unshard time).

v3 structure:
  - fp16 qk_rot / cos / sin / perm: fp32r scores matmuls ran at half PE
    clock (fp32_mode=HIGH); fp16 runs 1 row/cycle, and the two 64-deep
    head matmuls of a scores tile run concurrently in PE half-arrays.
  - causal diag masking via -1e9 mask matmuls accumulated into the scores
    psum (bf16 for range) - stays on the PE, off the exp critical path.
  - y_all -> yt transposes via SBUF->SBUF transpose DMA.
  - scalar engine runs only the softmax exp (the attention serial
    bottleneck); PSUM->SBUF copies on DVE.
  - software pipelining: qkv(j+1) and wo(j-1) groups interleaved with
    attention(j) tile steps; attention(0) overlaps qkv(0)'s tail.
  - batched input DMAs (31 issues): XT in column halves, wqkv|wv merged,
    cos|sin merged per chunk, all issued up front on SP in priority order.
"""
import sys

sys.path.insert(0, "/opt/trn_rl_repo")

import numpy as np

import concourse.bass as bass
import concourse.mybir as mybir
import concourse.tile as tile
from concourse import bacc, bass_utils

B, L, D = 2, 2048, 1024
NH, HD = 16, 64
NCORES = 8
HPC = 4            # heads per core
LQB = 512          # Lq block per S^T unit
NLQ = L // LQB     # 4
NLT = L // 128     # 16
KT = D // 128      # 8 contraction tiles for projections

F32 = mybir.dt.float32
F16 = mybir.dt.float16
BF16 = mybir.dt.bfloat16

_cache = {}


def build_nc(debug=False):
    nc = bacc.Bacc("TRN2", target_bir_lowering=False, debug=False)

    # XT column halves: a-half = cols 0..1023 (chunks 0,1), b-half = rest
    XT = nc.dram_tensor("XT", [D, L], F16, kind="ExternalInput")
    WQKV = nc.dram_tensor("WQKV", [D, 772], F16, kind="ExternalInput")
    WOT = nc.dram_tensor("WOT", [HPC * HD, D], F16, kind="ExternalInput")
    PERM = nc.dram_tensor("PERM", [128, 128], F16, kind="ExternalInput")
    CS = nc.dram_tensor("CS", [128, 2 * L], F16, kind="ExternalInput")
    MASKS = nc.dram_tensor("MASKS", [128, 256], BF16, kind="ExternalInput")
    OUT = nc.dram_tensor("OUT", [L, D], F32, kind="ExternalOutput")

    with tile.TileContext(nc) as tc:
        with (
            tc.tile_pool(name="consts", bufs=1) as cpool,
            tc.tile_pool(name="weights", bufs=1) as wpool,
            tc.tile_pool(name="qkrot", bufs=1) as rotpool,
            tc.tile_pool(name="vsb", bufs=1) as vpool,
            tc.tile_pool(name="yall", bufs=1) as ypool,
            tc.tile_pool(name="ytr", bufs=1) as ytpool,
            tc.tile_pool(name="xt", bufs=1) as xpool,
            tc.tile_pool(name="tmps", bufs=4) as tpool,
            tc.tile_pool(name="raws", bufs=3) as rawpool,
            tc.tile_pool(name="pts", bufs=4) as ptpool,
            tc.tile_pool(name="outsb", bufs=3) as opool,
            tc.tile_pool(name="recs", bufs=4) as recpool,
            tc.tile_pool(name="psB", bufs=3, space="PSUM") as bigpool,
            tc.tile_pool(name="psY", bufs=1, space="PSUM") as psypool,
        ):
            # ---- all input DMAs up front on SP, priority-ordered ---------
            # wqkv_sb[k]: [128, 772] = wq|wk (512, scaled q) then wv (260)
            wqkv_sb = [None] * KT
            xth = [[None] * KT for _ in range(2)]   # column halves
            cs_sb = [None] * NLQ                    # cos|sin per chunk

            def load_wqkv(k):
                w = wpool.tile([128, 772], F16, tag=f"wqkv{k}",
                               name=f"wqkv{k}")
                nc.sync.dma_start(w[:], WQKV[k * 128:(k + 1) * 128, :])
                wqkv_sb[k] = w

            def load_xt_half(hf, k):
                xs = slice(hf * 1024, (hf + 1) * 1024)
                t = xpool.tile([128, 1024], F16, tag=f"xt{hf}_{k}",
                               name=f"xt{hf}_{k}")
                nc.sync.dma_start(t[:], XT[k * 128:(k + 1) * 128, xs])
                xth[hf][k] = t

            def load_cs(j):
                t = cpool.tile([128, 1024], F16, tag=f"cs{j}")
                nc.sync.dma_start(t[:], CS[:, j * 1024:(j + 1) * 1024])
                cs_sb[j] = t

            perm_sb = cpool.tile([128, 128], F16, tag="perm")
            masks_sb = cpool.tile([128, 256], BF16, tag="masks")

            # chunk-0 critical path first
            for k in range(KT):
                load_wqkv(k)
                load_xt_half(0, k)
                if k == 0:
                    nc.sync.dma_start(perm_sb[:], PERM[:, :])
                    load_cs(0)
                if k == 4:
                    nc.sync.dma_start(masks_sb[:], MASKS[:, :])
            load_cs(1)
            wot_sb = []
            for c2 in range(2):
                w = wpool.tile([128, D], F16, tag=f"wo{c2}", name=f"wo{c2}")
                nc.sync.dma_start(w[:], WOT[c2 * 128:(c2 + 1) * 128, :])
                wot_sb.append(w)
            for k in range(KT):
                load_xt_half(1, k)
            load_cs(2)
            load_cs(3)

            def xt_sl(j, k):
                """AP for chunk j's 512 columns of x^T k-tile."""
                hf, o = divmod(j * LQB, 1024)
                return xth[hf][k][:, o:o + LQB]

            # persistent activation storage
            # qk_rot[m]: m=0,1 -> q head-pairs (h01, h23); m=2,3 -> k pairs
            qk_rot = [rotpool.tile([128, L], F16, tag=f"rot{m}",
                                   name=f"rot{m}")
                      for m in range(4)]
            v_sb = [vpool.tile([128, 260], F16, tag=f"v{t}", name=f"v{t}")
                    for t in range(NLT)]
            y_all = [ypool.tile([128, HPC * HD], F16, tag=f"y{i}", name=f"y{i}")
                     for i in range(NLT)]
            yt_sb = [ytpool.tile([128, L], F16, tag=f"yt{c2}", name=f"yt{c2}")
                     for c2 in range(2)]

            def qkv_m_group(j, m):
                xs = slice(j * LQB, (j + 1) * LQB)
                big = bigpool.tile([128, 1024], F32, tag="big", name="big")
                ps = big[:, 0:512]
                psw = big[:, 512:1024]
                for k in range(KT):
                    nc.tensor.matmul(
                        ps, wqkv_sb[k][:, m * 128:(m + 1) * 128], xt_sl(j, k),
                        start=(k == 0), stop=(k == KT - 1),
                    )
                raw = rawpool.tile([128, LQB], F16, tag="raw")
                nc.vector.tensor_copy(raw[:], ps)
                t1 = tpool.tile([128, LQB], F16, tag="t1")
                nc.vector.tensor_mul(t1[:], raw[:], cs_sb[j][:, 0:512])
                nc.tensor.matmul(psw, perm_sb[:], raw[:],
                                 start=True, stop=True)
                t2 = tpool.tile([128, LQB], F16, tag="t2")
                nc.vector.tensor_mul(t2[:], psw, cs_sb[j][:, 512:1024])
                nc.vector.tensor_add(qk_rot[m][:, xs], t1[:], t2[:])

            def v_group(j, i2):
                ti = j * 4 + i2
                bigv = bigpool.tile([128, 1024], F32, tag="big", name="bigv")
                psv = bigv[:, 0:260]
                for k in range(KT):
                    nc.tensor.matmul(
                        psv, xt_sl(j, k)[:, i2 * 128:(i2 + 1) * 128],
                        wqkv_sb[k][:, 512:772],
                        start=(k == 0), stop=(k == KT - 1),
                    )
                nc.vector.tensor_copy(v_sb[ti][:], psv)
                nc.vector.memset(v_sb[ti][:, 64:260:65], 1.0)

            # ---- attention ----------------------------------------------
            y_ps = [None, None]

            def wave_start(jq, hp):
                for h in range(2):
                    y_ps[h] = psypool.tile([128, 260], F32, tag=f"yps{h}",
                                           name=f"yps{h}", bufs=1)
                    nc.vector.memset(y_ps[h][:], 0.0)

            def attn_step(jq, hp, t):
                nt = 4 * jq + 4
                ks = slice(t * 128, (t + 1) * 128)
                diag = t >= 4 * jq
                off = max(0, t * 128 - jq * LQB)
                r = off // 128
                sp = bigpool.tile([128, 1024], F32, tag="big", name="sp")
                for h in range(2):
                    hs = slice(64 * h, 64 * h + 64)
                    nc.tensor.matmul(
                        sp[:, 512 * h + off:512 * h + 512],
                        qk_rot[2 + hp][hs, ks],
                        qk_rot[hp][hs, jq * LQB + off:(jq + 1) * LQB],
                        start=True, stop=not diag,
                    )
                if diag:
                    # accumulate -1e9 * strict-upper-tri into the diag block
                    for h in range(2):
                        nc.tensor.matmul(
                            sp[:, 512 * h + off:512 * h + off + 128],
                            masks_sb[:, 0:128],
                            masks_sb[:, 128:256],
                            start=False, stop=True,
                        )
                pt = ptpool.tile([128, 1024], F16, tag="pt")
                nc.scalar.activation(
                    pt[:, off:1024], sp[:, off:1024],
                    mybir.ActivationFunctionType.Exp
                )
                for h in range(2):
                    H = 2 * hp + h
                    for js in range(r, 4):
                        nc.tensor.matmul(
                            y_ps[h][:, 65 * js:65 * js + 65],
                            pt[:, 512 * h + 128 * js:512 * h + 128 * js + 128],
                            v_sb[t][:, 65 * H:65 * H + 65],
                            start=False, stop=(t == nt - 1),
                            skip_group_check=True,
                        )

            def wave_end(jq, hp):
                # normalize: y /= rowsum, write into y_all as fp16
                for h in range(2):
                    H = 2 * hp + h
                    rec = recpool.tile([128, 4], F32, tag="rec")
                    nc.vector.reciprocal(rec[:], y_ps[h][:, 64:260:65])
                    for js in range(4):
                        i = 4 * jq + js
                        nc.vector.tensor_scalar_mul(
                            y_all[i][:, HD * H:HD * H + HD],
                            y_ps[h][:, 65 * js:65 * js + 64],
                            rec[:, js:js + 1],
                        )

            def transpose_tiles(jq):
                # y_all[i] (L-rows x d) -> yt (d x L) via SBUF->SBUF
                # transpose DMA (replaces PE transposes + DVE copies)
                for i in range(4 * jq, 4 * jq + 4):
                    for c2 in range(2):
                        nc.sync.dma_start(
                            yt_sb[c2][:, 128 * i:128 * i + 128],
                            y_all[i][:, 128 * c2:128 * c2 + 128],
                            transpose=True,
                        )

            def wo_tile(i):
                po = bigpool.tile([128, 1024], F32, tag="big", name="po")
                for half in range(2):
                    for c2 in range(2):
                        nc.tensor.matmul(
                            po[:, 512 * half:512 * half + 512],
                            yt_sb[c2][:, 128 * i:128 * i + 128],
                            wot_sb[c2][:, 512 * half:512 * half + 512],
                            start=(c2 == 0), stop=(c2 == 1),
                        )
                ob = opool.tile([128, 1024], F32, tag="ob")
                nc.vector.tensor_copy(ob[:], po[:])
                nc.sync.dma_start(OUT[128 * i:128 * i + 128, :], ob[:])

            def attn_zip(jq, fillers, pre=()):
                """Emit attention(jq) steps with filler work interleaved so
                the PE stays fed during the exp-gated stretches. `pre` ops
                are required before wave hp=1 only (chunk-0 overlap)."""
                nt = 4 * jq + 4
                steps = []
                for hp in range(2):
                    if hp == 1:
                        for fn in pre:
                            steps.append(("fn", fn))
                    steps.append(("ws", hp))
                    for t in range(nt):
                        steps.append(("st", hp, t))
                    steps.append(("we", hp))
                st_idx = [i for i, s in enumerate(steps) if s[0] == "st"]
                ns, nf = len(st_idx), len(fillers)
                fpos = {}
                for f in range(nf):
                    pos = st_idx[min(ns - 1, (f * ns) // nf)]
                    fpos.setdefault(pos, []).append(fillers[f])
                for i, s in enumerate(steps):
                    for fn in fpos.get(i, ()):
                        fn()
                    if s[0] == "ws":
                        wave_start(jq, s[1])
                    elif s[0] == "st":
                        attn_step(jq, s[1], s[2])
                    elif s[0] == "we":
                        wave_end(jq, s[1])
                    else:
                        s[1]()

            # ---- main schedule ------------------------------------------
            # chunk 0: emit only the h01 q/k groups (m=0,2); wave hp=0 of
            # attention(0) can then start while m=1,3 and v emit as fillers.
            qkv_m_group(0, 0)
            qkv_m_group(0, 2)
            for j in range(NLQ):
                fillers = []
                pre = ()
                if j == 0:
                    fillers += [lambda: qkv_m_group(0, 1)]
                    fillers += [(lambda i2=i2: v_group(0, i2))
                                for i2 in range(4)]
                    pre = (lambda: qkv_m_group(0, 3),)
                if j < NLQ - 1:
                    jn = j + 1
                    fillers += [
                        (lambda m=m, jn=jn: qkv_m_group(jn, m))
                        for m in range(4)
                    ]
                    fillers += [
                        (lambda i2=i2, jn=jn: v_group(jn, i2))
                        for i2 in range(4)
                    ]
                if j == 1:
                    fillers += [(lambda i=i: wo_tile(i)) for i in range(0, 4)]
                if j == 3:
                    fillers += [(lambda i=i: wo_tile(i)) for i in range(4, 12)]
                attn_zip(j, fillers, pre)
                transpose_tiles(j)
            for i in range(12, 16):
                wo_tile(i)

    nc.finalize()
    return nc


def prep_inputs(x, wqkv, wo):
    """Build the 8 per-core input dicts from the full-problem inputs."""
    import ml_dtypes

    x = np.asarray(x, dtype=np.float32)
    wqkv = np.asarray(wqkv, dtype=np.float32)
    wo = np.asarray(wo, dtype=np.float32)

    # rope tables; CS layout: per chunk j a [cos_j (512) | sin_j (512)]
    # block at cols [1024j, 1024j+1024), loaded as one DMA per chunk.
    inv_freq = 1.0 / (10000.0 ** (np.arange(0, HD, 2, dtype=np.float32) / HD))
    t = np.arange(L, dtype=np.float32)
    freqs = np.outer(t, inv_freq)                  # (L, 32)
    cos32 = np.cos(freqs).T.astype(np.float32)     # (32, L)
    sin32 = np.sin(freqs).T.astype(np.float32)
    COS = np.tile(cos32, (4, 1))                                 # (128, L)
    SIN = np.concatenate([-sin32, sin32, -sin32, sin32], axis=0)
    CSfull = np.zeros((128, 2 * L), dtype=np.float32)
    for j in range(NLQ):
        CSfull[:, 1024 * j:1024 * j + 512] = COS[:, 512 * j:512 * j + 512]
        CSfull[:, 1024 * j + 512:1024 * j + 1024] = \
            SIN[:, 512 * j:512 * j + 512]

    # 32-block swap permutation (within each head's 64 rows)
    PERM = np.zeros((128, 128), dtype=np.float32)
    for blk in range(2):
        o = 64 * blk
        PERM[o:o + 32, o + 32:o + 64] = np.eye(32)
        PERM[o + 32:o + 64, o:o + 32] = np.eye(32)

    bf = ml_dtypes.bfloat16
    NEGI = (-1e9 * np.eye(128)).astype(np.float32)
    BIGM = (np.arange(128)[None, :] < np.arange(128)[:, None])
    MASKS = np.concatenate(
        [NEGI, BIGM.astype(np.float32)], axis=1).astype(bf)      # (128, 256)

    in_maps = []
    scale = np.float32(HD ** -0.5)
    for c in range(NCORES):
        b, g = divmod(c, 4)
        qrows = slice(256 * g, 256 * g + 256)
        krows = slice(1024 + 256 * g, 1024 + 256 * g + 256)
        vrows = slice(2048 + 256 * g, 2048 + 256 * g + 256)

        XT = np.ascontiguousarray(x[b].T)                        # (1024, 2048)
        wq = (wqkv[qrows, :] * scale).T                          # (1024, 256)
        wk = wqkv[krows, :].T
        vpart = wqkv[vrows, :].T                                 # (1024, 256)
        WV = np.zeros((D, 260), dtype=np.float32)
        for h in range(HPC):
            WV[:, 65 * h:65 * h + 64] = vpart[:, 64 * h:64 * h + 64]
        WQKV = np.ascontiguousarray(
            np.concatenate([wq, wk, WV], axis=1))                # (1024, 772)
        WOT = np.ascontiguousarray(wo[:, 256 * g:256 * g + 256].T)

        in_maps.append({
            "XT": XT.astype(np.float16),
            "WQKV": WQKV.astype(np.float16),
            "WOT": WOT.astype(np.float16),
            "CS": CSfull.astype(np.float16),
            "PERM": PERM.astype(np.float16),
            "MASKS": MASKS,
        })
    return in_maps


def kernel(x, wqkv, wo):
    if "nc" not in _cache:
        _cache["nc"] = build_nc()
    nc = _cache["nc"]
    in_maps = prep_inputs(x, wqkv, wo)
    res = bass_utils.run_bass_kernel_spmd(nc, in_maps, list(range(NCORES)))
    outs = [res.results[c]["OUT"] for c in range(NCORES)]
    out0 = outs[0] + outs[1] + outs[2] + outs[3]
    out1 = outs[4] + outs[5] + outs[6] + outs[7]
    return np.stack([out0, out1]).astype(np.float32)


# revision 10
# speedup vs baseline: 1.2224x; 1.0291x over previous
"""Multi-head self-attention (RoPE, causal) Bass kernel for 8 TRN2 NeuronCores.

Problem: x (2, 2048, 1024) f32, wqkv (3072, 1024), wo (1024, 1024).
  qkv = x @ wqkv.T ; RoPE(q, k) ; causal softmax attention (16 heads, hd=64);
  out = y @ wo.T.

Sharding: batch (2-way) x head-group (4-way) tensor parallel = 8 cores.
Each core computes a full (2048, 1024) partial output for its batch from its
4 heads; host sums the 4 partials per batch (the TP all-reduce done at
unshard time).

v4 structure:
  - fp16 throughout the rope/scores path (fp32r ran the PE at half clock;
    the paired 64-deep head matmuls share the PE as half-array tiles).
  - softmax exp is the serial bottleneck (~1us per 128x1024 tile on the
    scalar engine, ~85us total): the whole kernel is organized to keep the
    exp stream dense. A flattened (wave, tile) pipeline emits scores+exp
    ("front") LA tiles ahead of the P@V accumulation ("back"), flowing
    across wave/chunk boundaries without draining; qkv(j+1) and wo(j-1)
    matmul groups fill the PE between attention steps.
  - input DMA issue is spread across the idle SP/DVE/ACT sequencers
    (~650ns per issue each) so chunk-0 data lands in ~4us, not ~25us.
  - y_all -> yt transposes via SBUF->SBUF transpose DMA (PE transposes for
    the last chunk, where the PE is idle and the SP queue is the tail).
"""
import sys

sys.path.insert(0, "/opt/trn_rl_repo")

import numpy as np

import concourse.bass as bass
import concourse.mybir as mybir
import concourse.tile as tile
from concourse import bacc, bass_utils
from concourse.masks import make_identity

B, L, D = 2, 2048, 1024
NH, HD = 16, 64
NCORES = 8
HPC = 4            # heads per core
LQB = 512          # Lq block per S^T unit
NLQ = L // LQB     # 4
NLT = L // 128     # 16
KT = D // 128      # 8 contraction tiles for projections

F32 = mybir.dt.float32
F16 = mybir.dt.float16
BF16 = mybir.dt.bfloat16

_cache = {}


def build_nc(debug=False):
    nc = bacc.Bacc("TRN2", target_bir_lowering=False, debug=False)

    XT = nc.dram_tensor("XT", [D, L], F16, kind="ExternalInput")
    WQKV = nc.dram_tensor("WQKV", [D, 772], F16, kind="ExternalInput")
    WOT = nc.dram_tensor("WOT", [HPC * HD, D], F16, kind="ExternalInput")
    PERM = nc.dram_tensor("PERM", [128, 128], F16, kind="ExternalInput")
    CS = nc.dram_tensor("CS", [128, 2 * L], F16, kind="ExternalInput")
    MASKS = nc.dram_tensor("MASKS", [128, 256], BF16, kind="ExternalInput")
    OUT = nc.dram_tensor("OUT", [L, D], F32, kind="ExternalOutput")

    with tile.TileContext(nc) as tc:
        with (
            tc.tile_pool(name="consts", bufs=1) as cpool,
            tc.tile_pool(name="weights", bufs=1) as wpool,
            tc.tile_pool(name="qkrot", bufs=1) as rotpool,
            tc.tile_pool(name="vsb", bufs=1) as vpool,
            tc.tile_pool(name="yall", bufs=1) as ypool,
            tc.tile_pool(name="ytr", bufs=1) as ytpool,
            tc.tile_pool(name="xt", bufs=1) as xpool,
            tc.tile_pool(name="tmps", bufs=4) as tpool,
            tc.tile_pool(name="raws", bufs=3) as rawpool,
            tc.tile_pool(name="pts", bufs=4) as ptpool,
            tc.tile_pool(name="outsb", bufs=3) as opool,
            tc.tile_pool(name="recs", bufs=4) as recpool,
            tc.tile_pool(name="psB", bufs=3, space="PSUM") as bigpool,
            tc.tile_pool(name="psY", bufs=1, space="PSUM") as psypool,
        ):
            # ---- input DMAs, split across idle sequencers ----------------
            wqkv_sb = [None] * KT
            xth = [[None] * KT for _ in range(2)]   # column halves
            cs_sb = [None] * NLQ                    # cos|sin per chunk

            perm_sb = cpool.tile([128, 128], F16, tag="perm")
            masks_sb = cpool.tile([128, 256], BF16, tag="masks")
            idn = cpool.tile([128, 128], F16, tag="idn")

            # SP: wqkv (chunk-0 critical), then masks/wo, then xt b-halves
            for k in range(KT):
                w = wpool.tile([128, 772], F16, tag=f"wqkv{k}",
                               name=f"wqkv{k}")
                nc.sync.dma_start(w[:], WQKV[k * 128:(k + 1) * 128, :])
                wqkv_sb[k] = w
            # gpsimd (SWDGE, otherwise idle): xt a-halves (chunks 0,1)
            for k in range(KT):
                t = xpool.tile([128, 1024], F16, tag=f"xt0_{k}",
                               name=f"xt0_{k}")
                nc.gpsimd.dma_start(t[:], XT[k * 128:(k + 1) * 128, 0:1024])
                xth[0][k] = t
            # ACT: perm + cos/sin chunks
            nc.scalar.dma_start(perm_sb[:], PERM[:, :])
            for j in range(NLQ):
                c = cpool.tile([128, 1024], F16, tag=f"cs{j}")
                nc.scalar.dma_start(c[:], CS[:, j * 1024:(j + 1) * 1024])
                cs_sb[j] = c
            # SP tail: masks, wo, xt b-halves (needed from ~35us)
            nc.sync.dma_start(masks_sb[:], MASKS[:, :])
            wot_sb = []
            for c2 in range(2):
                w = wpool.tile([128, D], F16, tag=f"wo{c2}", name=f"wo{c2}")
                nc.sync.dma_start(w[:], WOT[c2 * 128:(c2 + 1) * 128, :])
                wot_sb.append(w)
            for k in range(KT):
                t = xpool.tile([128, 1024], F16, tag=f"xt1_{k}",
                               name=f"xt1_{k}")
                nc.sync.dma_start(t[:], XT[k * 128:(k + 1) * 128, 1024:2048])
                xth[1][k] = t
            make_identity(nc, idn[:])

            def xt_sl(j, k):
                """AP for chunk j's 512 columns of x^T k-tile."""
                hf, o = divmod(j * LQB, 1024)
                return xth[hf][k][:, o:o + LQB]

            # persistent activation storage
            # qk_rot[m]: m=0,1 -> q head-pairs (h01, h23); m=2,3 -> k pairs
            qk_rot = [rotpool.tile([128, L], F16, tag=f"rot{m}",
                                   name=f"rot{m}")
                      for m in range(4)]
            v_sb = [vpool.tile([128, 260], F16, tag=f"v{t}", name=f"v{t}")
                    for t in range(NLT)]
            y_all = [ypool.tile([128, HPC * HD], F16, tag=f"y{i}", name=f"y{i}")
                     for i in range(NLT)]
            yt_sb = [ytpool.tile([128, L], F16, tag=f"yt{c2}", name=f"yt{c2}")
                     for c2 in range(2)]

            def qkv_m_group(j, m):
                xs = slice(j * LQB, (j + 1) * LQB)
                big = bigpool.tile([128, 1024], F32, tag="big", name="big")
                ps = big[:, 0:512]
                psw = big[:, 512:1024]
                for k in range(KT):
                    nc.tensor.matmul(
                        ps, wqkv_sb[k][:, m * 128:(m + 1) * 128], xt_sl(j, k),
                        start=(k == 0), stop=(k == KT - 1),
                    )
                raw = rawpool.tile([128, LQB], F16, tag="raw")
                nc.vector.tensor_copy(raw[:], ps)
                t1 = tpool.tile([128, LQB], F16, tag="t1")
                nc.vector.tensor_mul(t1[:], raw[:], cs_sb[j][:, 0:512])
                nc.tensor.matmul(psw, perm_sb[:], raw[:],
                                 start=True, stop=True)
                t2 = tpool.tile([128, LQB], F16, tag="t2")
                nc.vector.tensor_mul(t2[:], psw, cs_sb[j][:, 512:1024])
                nc.vector.tensor_add(qk_rot[m][:, xs], t1[:], t2[:])

            def v_group(j, i2):
                ti = j * 4 + i2
                bigv = bigpool.tile([128, 1024], F32, tag="big", name="bigv")
                psv = bigv[:, 0:260]
                for k in range(KT):
                    nc.tensor.matmul(
                        psv, xt_sl(j, k)[:, i2 * 128:(i2 + 1) * 128],
                        wqkv_sb[k][:, 512:772],
                        start=(k == 0), stop=(k == KT - 1),
                    )
                nc.vector.tensor_copy(v_sb[ti][:], psv)
                nc.vector.memset(v_sb[ti][:, 64:260:65], 1.0)

            # ---- attention: front (scores+exp) / back (P@V) pipeline ----
            y_ps = {}       # wave -> [tile, tile]
            pt_of = {}      # (wave, t) -> pt tile

            def front(w, t):
                jq, hp = w
                ks = slice(t * 128, (t + 1) * 128)
                diag = t >= 4 * jq
                off = max(0, t * 128 - jq * LQB)
                sp = bigpool.tile([128, 1024], F32, tag="big", name="sp")
                for h in range(2):
                    hs = slice(64 * h, 64 * h + 64)
                    nc.tensor.matmul(
                        sp[:, 512 * h + off:512 * h + 512],
                        qk_rot[2 + hp][hs, ks],
                        qk_rot[hp][hs, jq * LQB + off:(jq + 1) * LQB],
                        start=True, stop=not diag,
                    )
                if diag:
                    # accumulate -1e9 * strict-upper-tri into the diag block
                    for h in range(2):
                        nc.tensor.matmul(
                            sp[:, 512 * h + off:512 * h + off + 128],
                            masks_sb[:, 0:128],
                            masks_sb[:, 128:256],
                            start=False, stop=True,
                        )
                pt = ptpool.tile([128, 1024], F16, tag="pt")
                nc.scalar.activation(
                    pt[:, off:1024], sp[:, off:1024],
                    mybir.ActivationFunctionType.Exp
                )
                pt_of[(w, t)] = pt

            def wave_start(w):
                y_ps[w] = []
                for h in range(2):
                    yp = psypool.tile([128, 260], F32, tag=f"yps{h}",
                                      name=f"yps{h}", bufs=1)
                    nc.vector.memset(yp[:], 0.0)
                    y_ps[w].append(yp)

            def back(w, t):
                jq, hp = w
                nt = 4 * jq + 4
                off = max(0, t * 128 - jq * LQB)
                r = off // 128
                pt = pt_of.pop((w, t))
                for h in range(2):
                    H = 2 * hp + h
                    for js in range(r, 4):
                        nc.tensor.matmul(
                            y_ps[w][h][:, 65 * js:65 * js + 65],
                            pt[:, 512 * h + 128 * js:512 * h + 128 * js + 128],
                            v_sb[t][:, 65 * H:65 * H + 65],
                            start=False, stop=(t == nt - 1),
                            skip_group_check=True,
                        )

            def wave_end(w):
                jq, hp = w
                for h in range(2):
                    H = 2 * hp + h
                    rec = recpool.tile([128, 4], F32, tag="rec")
                    nc.vector.reciprocal(rec[:], y_ps[w][h][:, 64:260:65])
                    for js in range(4):
                        i = 4 * jq + js
                        nc.vector.tensor_scalar_mul(
                            y_all[i][:, HD * H:HD * H + HD],
                            y_ps[w][h][:, 65 * js:65 * js + 64],
                            rec[:, js:js + 1],
                        )
                del y_ps[w]

            def transpose_tiles(jq):
                # y_all[i] (L-rows x d) -> yt (d x L).  SBUF->SBUF transpose
                # DMA for early chunks; PE transposes for the last chunk
                # (PE idle at the tail, SP queue is the critical path there).
                for i in range(4 * jq, 4 * jq + 4):
                    if jq < 3:
                        for c2 in range(2):
                            nc.sync.dma_start(
                                yt_sb[c2][:, 128 * i:128 * i + 128],
                                y_all[i][:, 128 * c2:128 * c2 + 128],
                                transpose=True,
                            )
                    else:
                        bigt = bigpool.tile([128, 1024], F32, tag="big",
                                            name="bigt")
                        tp16 = bigt[:].bitcast(F16)
                        for c2 in range(2):
                            nc.tensor.transpose(
                                tp16[:, 128 * c2:128 * c2 + 128],
                                y_all[i][:, 128 * c2:128 * c2 + 128],
                                idn[:],
                            )
                            nc.vector.tensor_copy(
                                yt_sb[c2][:, 128 * i:128 * i + 128],
                                tp16[:, 128 * c2:128 * c2 + 128],
                            )

            def wo_tile(i):
                po = bigpool.tile([128, 1024], F32, tag="big", name="po")
                for half in range(2):
                    for c2 in range(2):
                        nc.tensor.matmul(
                            po[:, 512 * half:512 * half + 512],
                            yt_sb[c2][:, 128 * i:128 * i + 128],
                            wot_sb[c2][:, 512 * half:512 * half + 512],
                            start=(c2 == 0), stop=(c2 == 1),
                        )
                ob = opool.tile([128, 1024], F32, tag="ob")
                nc.vector.tensor_copy(ob[:], po[:])
                nc.sync.dma_start(OUT[128 * i:128 * i + 128, :], ob[:])

            # ---- main schedule: flattened lookahead pipeline ------------
            LA = 2
            waves = [(jq, hp) for jq in range(NLQ) for hp in range(2)]
            nts = {w: 4 * w[0] + 4 for w in waves}
            seq = [(w, t) for w in waves for t in range(nts[w])]

            # chunk-0 head-pair 01 q/k groups first; everything else is
            # filler work woven between front steps of each window.
            win_fillers = {
                0: [lambda: v_group(0, 0), lambda: qkv_m_group(0, 1),
                    lambda: v_group(0, 1), lambda: qkv_m_group(0, 3),
                    lambda: v_group(0, 2), lambda: v_group(0, 3)]
                   + [(lambda m=m: qkv_m_group(1, m)) for m in range(4)]
                   + [(lambda i2=i2: v_group(1, i2)) for i2 in range(4)],
                1: [(lambda m=m: qkv_m_group(2, m)) for m in range(4)]
                   + [(lambda i2=i2: v_group(2, i2)) for i2 in range(4)]
                   + [(lambda i=i: wo_tile(i)) for i in range(0, 4)],
                2: [(lambda m=m: qkv_m_group(3, m)) for m in range(4)]
                   + [(lambda i2=i2: v_group(3, i2)) for i2 in range(4)],
                3: [(lambda i=i: wo_tile(i)) for i in range(4, 12)],
            }
            # spread each window's fillers over its front positions
            win_pos = {jq: [p for p, (w, t) in enumerate(seq) if w[0] == jq]
                       for jq in range(NLQ)}
            fpos = {}
            for jq, fl in win_fillers.items():
                pos, nf = win_pos[jq], len(fl)
                for f in range(nf):
                    p = pos[min(len(pos) - 1, (f * len(pos)) // nf)]
                    fpos.setdefault(p, []).append(fl[f])

            def emit_back(q):
                wq, tq = seq[q]
                if tq == 0:
                    wave_start(wq)
                back(wq, tq)
                if tq == nts[wq] - 1:
                    wave_end(wq)
                    if wq[1] == 1:
                        transpose_tiles(wq[0])

            qkv_m_group(0, 0)
            qkv_m_group(0, 2)
            for p, (w, t) in enumerate(seq):
                for fn in fpos.get(p, ()):
                    fn()
                front(w, t)
                if p - LA >= 0:
                    emit_back(p - LA)
            for q in range(len(seq) - LA, len(seq)):
                emit_back(q)
            for i in range(12, 16):
                wo_tile(i)

    nc.finalize()
    return nc


def prep_inputs(x, wqkv, wo):
    """Build the 8 per-core input dicts from the full-problem inputs."""
    import ml_dtypes

    x = np.asarray(x, dtype=np.float32)
    wqkv = np.asarray(wqkv, dtype=np.float32)
    wo = np.asarray(wo, dtype=np.float32)

    # rope tables; CS layout: per chunk j a [cos_j (512) | sin_j (512)]
    # block at cols [1024j, 1024j+1024), loaded as one DMA per chunk.
    inv_freq = 1.0 / (10000.0 ** (np.arange(0, HD, 2, dtype=np.float32) / HD))
    t = np.arange(L, dtype=np.float32)
    freqs = np.outer(t, inv_freq)                  # (L, 32)
    cos32 = np.cos(freqs).T.astype(np.float32)     # (32, L)
    sin32 = np.sin(freqs).T.astype(np.float32)
    COS = np.tile(cos32, (4, 1))                                 # (128, L)
    SIN = np.concatenate([-sin32, sin32, -sin32, sin32], axis=0)
    CSfull = np.zeros((128, 2 * L), dtype=np.float32)
    for j in range(NLQ):
        CSfull[:, 1024 * j:1024 * j + 512] = COS[:, 512 * j:512 * j + 512]
        CSfull[:, 1024 * j + 512:1024 * j + 1024] = \
            SIN[:, 512 * j:512 * j + 512]

    # 32-block swap permutation (within each head's 64 rows)
    PERM = np.zeros((128, 128), dtype=np.float32)
    for blk in range(2):
        o = 64 * blk
        PERM[o:o + 32, o + 32:o + 64] = np.eye(32)
        PERM[o + 32:o + 64, o:o + 32] = np.eye(32)

    bf = ml_dtypes.bfloat16
    NEGI = (-1e9 * np.eye(128)).astype(np.float32)
    BIGM = (np.arange(128)[None, :] < np.arange(128)[:, None])
    MASKS = np.concatenate(
        [NEGI, BIGM.astype(np.float32)], axis=1).astype(bf)      # (128, 256)

    in_maps = []
    scale = np.float32(HD ** -0.5)
    for c in range(NCORES):
        b, g = divmod(c, 4)
        qrows = slice(256 * g, 256 * g + 256)
        krows = slice(1024 + 256 * g, 1024 + 256 * g + 256)
        vrows = slice(2048 + 256 * g, 2048 + 256 * g + 256)

        XT = np.ascontiguousarray(x[b].T)                        # (1024, 2048)
        wq = (wqkv[qrows, :] * scale).T                          # (1024, 256)
        wk = wqkv[krows, :].T
        vpart = wqkv[vrows, :].T                                 # (1024, 256)
        WV = np.zeros((D, 260), dtype=np.float32)
        for h in range(HPC):
            WV[:, 65 * h:65 * h + 64] = vpart[:, 64 * h:64 * h + 64]
        WQKV = np.ascontiguousarray(
            np.concatenate([wq, wk, WV], axis=1))                # (1024, 772)
        WOT = np.ascontiguousarray(wo[:, 256 * g:256 * g + 256].T)

        in_maps.append({
            "XT": XT.astype(np.float16),
            "WQKV": WQKV.astype(np.float16),
            "WOT": WOT.astype(np.float16),
            "CS": CSfull.astype(np.float16),
            "PERM": PERM.astype(np.float16),
            "MASKS": MASKS,
        })
    return in_maps


def kernel(x, wqkv, wo):
    if "nc" not in _cache:
        _cache["nc"] = build_nc()
    nc = _cache["nc"]
    in_maps = prep_inputs(x, wqkv, wo)
    res = bass_utils.run_bass_kernel_spmd(nc, in_maps, list(range(NCORES)))
    outs = [res.results[c]["OUT"] for c in range(NCORES)]
    out0 = outs[0] + outs[1] + outs[2] + outs[3]
    out1 = outs[4] + outs[5] + outs[6] + outs[7]
    return np.stack([out0, out1]).astype(np.float32)


# revision 15
# speedup vs baseline: 1.2374x; 1.0123x over previous
"""Multi-head self-attention (RoPE, causal) Bass kernel for 8 TRN2 NeuronCores.

Problem: x (2, 2048, 1024) f32, wqkv (3072, 1024), wo (1024, 1024).
  qkv = x @ wqkv.T ; RoPE(q, k) ; causal softmax attention (16 heads, hd=64);
  out = y @ wo.T.

Sharding: batch (2-way) x head-group (4-way) tensor parallel = 8 cores.
Each core computes a full (2048, 1024) partial output for its batch from its
4 heads; host sums the 4 partials per batch (bf16 partials, f32 host sum).

v5 structure:
  - softmax exp on the scalar engine (~84us serial) is the bottleneck; the
    kernel is one flattened (wave, tile) pipeline that keeps the exp stream
    dense: scores+exp ("front") run LA tiles ahead of P@V ("back"), and
    qkv/wo groups fill the PE at deadline-chosen front positions.
  - startup: WQKV columns reordered [q01|k01|q23|k23|v] and loaded in an
    A (q01/k01) + B split so the first attention wave needs only ~1.5MB;
    xt loaded per 512-col chunk (chunks 0,1 via gpsimd SWDGE, 2,3 late on
    SP); cos/sin/perm on the scalar sequencer.
  - fp16 everywhere in the rope/scores path (fp32r runs the PE at half
    clock); bf16 -1e9 mask matmuls for the causal diagonal; fp16 P and V
    with an ones-column accumulating the softmax denominator.
  - first PV matmul of each wave uses start=True (psum bank zero) instead
    of DVE memsets; y_all -> yt via transpose DMA (PE transposes for the
    last chunk); chunk-0 PSUM->SBUF copies on the then-idle scalar engine.
"""
import sys

sys.path.insert(0, "/opt/trn_rl_repo")

import numpy as np

import concourse.bass as bass
import concourse.mybir as mybir
import concourse.tile as tile
from concourse import bacc, bass_utils
from concourse.masks import make_identity

B, L, D = 2, 2048, 1024
NH, HD = 16, 64
NCORES = 8
HPC = 4            # heads per core
LQB = 512          # Lq block per S^T unit
NLQ = L // LQB     # 4
NLT = L // 128     # 16
KT = D // 128      # 8 contraction tiles for projections

F32 = mybir.dt.float32
F16 = mybir.dt.float16
BF16 = mybir.dt.bfloat16

# WQKV column offset per rope group m (m=0,1: q pairs h01,h23; 2,3: k pairs)
MCOL = {0: 0, 2: 128, 1: 256, 3: 384}

_cache = {}


def build_nc(debug=False):
    nc = bacc.Bacc("TRN2", target_bir_lowering=False, debug=False)

    XT = nc.dram_tensor("XT", [D, L], F16, kind="ExternalInput")
    WQKV = nc.dram_tensor("WQKV", [D, 772], F16, kind="ExternalInput")
    WOT = nc.dram_tensor("WOT", [HPC * HD, D], F16, kind="ExternalInput")
    PERM = nc.dram_tensor("PERM", [128, 128], F16, kind="ExternalInput")
    CS = nc.dram_tensor("CS", [128, 2 * L], F16, kind="ExternalInput")
    MASKS = nc.dram_tensor("MASKS", [128, 256], BF16, kind="ExternalInput")
    OUT = nc.dram_tensor("OUT", [L, D], BF16, kind="ExternalOutput")

    with tile.TileContext(nc) as tc:
        with (
            tc.tile_pool(name="consts", bufs=1) as cpool,
            tc.tile_pool(name="weights", bufs=1) as wpool,
            tc.tile_pool(name="qkrot", bufs=1) as rotpool,
            tc.tile_pool(name="vsb", bufs=1) as vpool,
            tc.tile_pool(name="yall", bufs=1) as ypool,
            tc.tile_pool(name="ytr", bufs=1) as ytpool,
            tc.tile_pool(name="xt", bufs=1) as xpool,
            tc.tile_pool(name="tmps", bufs=4) as tpool,
            tc.tile_pool(name="raws", bufs=3) as rawpool,
            tc.tile_pool(name="pts", bufs=4) as ptpool,
            tc.tile_pool(name="outsb", bufs=3) as opool,
            tc.tile_pool(name="recs", bufs=4) as recpool,
            tc.tile_pool(name="psB", bufs=3, space="PSUM") as bigpool,
            tc.tile_pool(name="psY", bufs=1, space="PSUM") as psypool,
        ):
            # ---- input DMAs, split across idle sequencers ----------------
            wqkv_sb = [None] * KT
            xts = [[None] * KT for _ in range(NLQ)]
            cs_sb = [None] * NLQ

            perm_sb = cpool.tile([128, 128], F16, tag="perm")
            masks_sb = cpool.tile([128, 256], BF16, tag="masks")
            idn = cpool.tile([128, 128], F16, tag="idn")

            # SP: wqkv A-halves (q01|k01 - the wave-(0,*) critical columns)
            for k in range(KT):
                wqkv_sb[k] = wpool.tile([128, 772], F16, tag=f"wqkv{k}",
                                        name=f"wqkv{k}")
                nc.sync.dma_start(wqkv_sb[k][:, 0:256],
                                  WQKV[k * 128:(k + 1) * 128, 0:256])
            # gpsimd (SWDGE): xt chunks 0, 1
            for j in range(2):
                for k in range(KT):
                    t = xpool.tile([128, LQB], F16, tag=f"xt{j}_{k}",
                                   name=f"xt{j}_{k}")
                    nc.gpsimd.dma_start(
                        t[:], XT[k * 128:(k + 1) * 128,
                                 j * LQB:(j + 1) * LQB])
                    xts[j][k] = t
            # ACT: perm + cos/sin chunks
            nc.scalar.dma_start(perm_sb[:], PERM[:, :])
            for j in range(NLQ):
                c = cpool.tile([128, 1024], F16, tag=f"cs{j}")
                nc.scalar.dma_start(c[:], CS[:, j * 1024:(j + 1) * 1024])
                cs_sb[j] = c
            # SP tail: wqkv B-halves, masks, wo, xt chunks 2, 3
            for k in range(KT):
                nc.sync.dma_start(wqkv_sb[k][:, 256:772],
                                  WQKV[k * 128:(k + 1) * 128, 256:772])
            nc.sync.dma_start(masks_sb[:], MASKS[:, :])
            wot_sb = []
            for c2 in range(2):
                w = wpool.tile([128, D], F16, tag=f"wo{c2}", name=f"wo{c2}")
                nc.sync.dma_start(w[:], WOT[c2 * 128:(c2 + 1) * 128, :])
                wot_sb.append(w)
            for j in range(2, NLQ):
                for k in range(KT):
                    t = xpool.tile([128, LQB], F16, tag=f"xt{j}_{k}",
                                   name=f"xt{j}_{k}")
                    nc.sync.dma_start(
                        t[:], XT[k * 128:(k + 1) * 128,
                                 j * LQB:(j + 1) * LQB])
                    xts[j][k] = t
            make_identity(nc, idn[:])

            # persistent activation storage
            qk_rot = [rotpool.tile([128, L], F16, tag=f"rot{m}",
                                   name=f"rot{m}")
                      for m in range(4)]
            v_sb = [vpool.tile([128, 260], F16, tag=f"v{t}", name=f"v{t}")
                    for t in range(NLT)]
            y_all = [ypool.tile([128, HPC * HD], F16, tag=f"y{i}", name=f"y{i}")
                     for i in range(NLT)]
            yt_sb = [ytpool.tile([128, L], F16, tag=f"yt{c2}", name=f"yt{c2}")
                     for c2 in range(2)]

            def ps_copy(j, out, in_):
                # chunk-0 copies ride the then-idle scalar engine
                if j == 0:
                    nc.scalar.copy(out, in_)
                else:
                    nc.vector.tensor_copy(out, in_)

            def qkv_m_group(j, m):
                xs = slice(j * LQB, (j + 1) * LQB)
                big = bigpool.tile([128, 1024], F32, tag="big", name="big")
                ps = big[:, 0:512]
                psw = big[:, 512:1024]
                co = MCOL[m]
                for k in range(KT):
                    nc.tensor.matmul(
                        ps, wqkv_sb[k][:, co:co + 128], xts[j][k][:],
                        start=(k == 0), stop=(k == KT - 1),
                    )
                raw = rawpool.tile([128, LQB], F16, tag="raw")
                ps_copy(j, raw[:], ps)
                t1 = tpool.tile([128, LQB], F16, tag="t1")
                nc.vector.tensor_mul(t1[:], raw[:], cs_sb[j][:, 0:512])
                nc.tensor.matmul(psw, perm_sb[:], raw[:],
                                 start=True, stop=True)
                t2 = tpool.tile([128, LQB], F16, tag="t2")
                nc.vector.tensor_mul(t2[:], psw, cs_sb[j][:, 512:1024])
                nc.vector.tensor_add(qk_rot[m][:, xs], t1[:], t2[:])

            def v_group(j, i2):
                ti = j * 4 + i2
                bigv = bigpool.tile([128, 1024], F32, tag="big", name="bigv")
                psv = bigv[:, 0:260]
                for k in range(KT):
                    nc.tensor.matmul(
                        psv, xts[j][k][:, i2 * 128:(i2 + 1) * 128],
                        wqkv_sb[k][:, 512:772],
                        start=(k == 0), stop=(k == KT - 1),
                    )
                ps_copy(j, v_sb[ti][:], psv)
                nc.vector.memset(v_sb[ti][:, 64:260:65], 1.0)

            # ---- attention: front (scores+exp) / back (P@V) pipeline ----
            y_ps = {}       # wave -> [tile, tile]
            pt_of = {}      # (wave, t) -> pt tile

            def front(w, t):
                jq, hp = w
                ks = slice(t * 128, (t + 1) * 128)
                diag = t >= 4 * jq
                off = max(0, t * 128 - jq * LQB)
                sp = bigpool.tile([128, 1024], F32, tag="big", name="sp")
                for h in range(2):
                    hs = slice(64 * h, 64 * h + 64)
                    nc.tensor.matmul(
                        sp[:, 512 * h + off:512 * h + 512],
                        qk_rot[2 + hp][hs, ks],
                        qk_rot[hp][hs, jq * LQB + off:(jq + 1) * LQB],
                        start=True, stop=not diag,
                    )
                if diag:
                    for h in range(2):
                        nc.tensor.matmul(
                            sp[:, 512 * h + off:512 * h + off + 128],
                            masks_sb[:, 0:128],
                            masks_sb[:, 128:256],
                            start=False, stop=True,
                        )
                pt = ptpool.tile([128, 1024], F16, tag="pt")
                nc.scalar.activation(
                    pt[:, off:1024], sp[:, off:1024],
                    mybir.ActivationFunctionType.Exp
                )
                pt_of[(w, t)] = pt

            def back(w, t):
                jq, hp = w
                nt = 4 * jq + 4
                off = max(0, t * 128 - jq * LQB)
                r = off // 128
                if t == 0:
                    y_ps[w] = [psypool.tile([128, 260], F32, tag=f"yps{h}",
                                            name=f"yps{h}", bufs=1)
                               for h in range(2)]
                pt = pt_of.pop((w, t))
                for h in range(2):
                    H = 2 * hp + h
                    for js in range(r, 4):
                        nc.tensor.matmul(
                            y_ps[w][h][:, 65 * js:65 * js + 65],
                            pt[:, 512 * h + 128 * js:512 * h + 128 * js + 128],
                            v_sb[t][:, 65 * H:65 * H + 65],
                            # first matmul of the wave zeroes the whole
                            # psum bank (the tile owns it); PE stream order
                            # guarantees it runs before the others
                            start=(t == 0 and js == 0), stop=(t == nt - 1),
                            skip_group_check=True,
                        )

            def wave_end(w):
                jq, hp = w
                for h in range(2):
                    H = 2 * hp + h
                    rec = recpool.tile([128, 4], F32, tag="rec")
                    nc.vector.reciprocal(rec[:], y_ps[w][h][:, 64:260:65])
                    for js in range(4):
                        i = 4 * jq + js
                        nc.vector.tensor_scalar_mul(
                            y_all[i][:, HD * H:HD * H + HD],
                            y_ps[w][h][:, 65 * js:65 * js + 64],
                            rec[:, js:js + 1],
                        )
                del y_ps[w]

            def transpose_tiles(jq):
                for i in range(4 * jq, 4 * jq + 4):
                    if jq < 3:
                        for c2 in range(2):
                            nc.sync.dma_start(
                                yt_sb[c2][:, 128 * i:128 * i + 128],
                                y_all[i][:, 128 * c2:128 * c2 + 128],
                                transpose=True,
                            )
                    else:
                        bigt = bigpool.tile([128, 1024], F32, tag="big",
                                            name="bigt")
                        tp16 = bigt[:].bitcast(F16)
                        for c2 in range(2):
                            nc.tensor.transpose(
                                tp16[:, 128 * c2:128 * c2 + 128],
                                y_all[i][:, 128 * c2:128 * c2 + 128],
                                idn[:],
                            )
                            nc.vector.tensor_copy(
                                yt_sb[c2][:, 128 * i:128 * i + 128],
                                tp16[:, 128 * c2:128 * c2 + 128],
                            )

            def wo_tile(i):
                po = bigpool.tile([128, 1024], F32, tag="big", name="po")
                for half in range(2):
                    for c2 in range(2):
                        nc.tensor.matmul(
                            po[:, 512 * half:512 * half + 512],
                            yt_sb[c2][:, 128 * i:128 * i + 128],
                            wot_sb[c2][:, 512 * half:512 * half + 512],
                            start=(c2 == 0), stop=(c2 == 1),
                        )
                ob = opool.tile([128, 1024], BF16, tag="ob")
                nc.vector.tensor_copy(ob[:], po[:])
                nc.sync.dma_start(OUT[128 * i:128 * i + 128, :], ob[:])

            # ---- main schedule: flattened lookahead pipeline ------------
            LA = 2
            waves = [(jq, hp) for jq in range(NLQ) for hp in range(2)]
            nts = {w: 4 * w[0] + 4 for w in waves}
            seq = [(w, t) for w in waves for t in range(nts[w])]
            # window jq -> offset of its first front position
            woff = {}
            p = 0
            for w in waves:
                if w[1] == 0:
                    woff[w[0]] = p
                p += nts[w]

            def M(j, m):
                return lambda: qkv_m_group(j, m)

            def V(j, i2):
                return lambda: v_group(j, i2)

            def WO(i):
                return lambda: wo_tile(i)

            # explicit deadline-aware filler positions (window-relative)
            win_fillers = {
                0: [(0, V(0, 0)), (1, M(0, 1)), (2, V(0, 1)), (3, M(0, 3)),
                    (4, V(0, 2)), (5, V(0, 3)), (6, M(1, 0)), (7, M(1, 2))],
                1: [(0, M(1, 1)), (2, V(1, 0)), (4, M(1, 3)), (5, V(1, 1)),
                    (6, V(1, 2)), (7, V(1, 3)), (10, M(2, 0)), (12, M(2, 2))],
                2: [(0, M(2, 1)), (2, V(2, 0)), (4, M(2, 3)), (6, V(2, 1)),
                    (8, V(2, 2)), (10, V(2, 3)), (12, M(3, 0)), (14, M(3, 2)),
                    (16, WO(0)), (18, WO(1)), (20, WO(2)), (22, WO(3))],
                3: [(0, M(3, 1)), (2, V(3, 0)), (4, M(3, 3)), (6, V(3, 1)),
                    (8, V(3, 2)), (10, V(3, 3)),
                    (14, WO(4)), (16, WO(5)), (18, WO(6)), (20, WO(7)),
                    (22, WO(8)), (24, WO(9)), (26, WO(10)), (28, WO(11))],
            }
            fpos = {}
            for jq, fl in win_fillers.items():
                for rel, fn in fl:
                    fpos.setdefault(woff[jq] + rel, []).append(fn)

            def emit_back(q):
                wq, tq = seq[q]
                back(wq, tq)
                if tq == nts[wq] - 1:
                    wave_end(wq)
                    if wq[1] == 1:
                        transpose_tiles(wq[0])

            qkv_m_group(0, 0)
            qkv_m_group(0, 2)
            for p, (w, t) in enumerate(seq):
                for fn in fpos.get(p, ()):
                    fn()
                front(w, t)
                if p - LA >= 0:
                    emit_back(p - LA)
            for q in range(len(seq) - LA, len(seq)):
                emit_back(q)
            for i in range(12, 16):
                wo_tile(i)

    nc.finalize()
    return nc


def prep_inputs(x, wqkv, wo):
    """Build the 8 per-core input dicts from the full-problem inputs."""
    import ml_dtypes

    x = np.asarray(x, dtype=np.float32)
    wqkv = np.asarray(wqkv, dtype=np.float32)
    wo = np.asarray(wo, dtype=np.float32)

    # rope tables; CS: per chunk j a [cos_j (512) | sin_j (512)] block
    inv_freq = 1.0 / (10000.0 ** (np.arange(0, HD, 2, dtype=np.float32) / HD))
    t = np.arange(L, dtype=np.float32)
    freqs = np.outer(t, inv_freq)                  # (L, 32)
    cos32 = np.cos(freqs).T.astype(np.float32)     # (32, L)
    sin32 = np.sin(freqs).T.astype(np.float32)
    COS = np.tile(cos32, (4, 1))                                 # (128, L)
    SIN = np.concatenate([-sin32, sin32, -sin32, sin32], axis=0)
    CSfull = np.zeros((128, 2 * L), dtype=np.float32)
    for j in range(NLQ):
        CSfull[:, 1024 * j:1024 * j + 512] = COS[:, 512 * j:512 * j + 512]
        CSfull[:, 1024 * j + 512:1024 * j + 1024] = \
            SIN[:, 512 * j:512 * j + 512]

    # 32-block swap permutation (within each head's 64 rows)
    PERM = np.zeros((128, 128), dtype=np.float32)
    for blk in range(2):
        o = 64 * blk
        PERM[o:o + 32, o + 32:o + 64] = np.eye(32)
        PERM[o + 32:o + 64, o:o + 32] = np.eye(32)

    bf = ml_dtypes.bfloat16
    NEGI = (-1e9 * np.eye(128)).astype(np.float32)
    BIGM = (np.arange(128)[None, :] < np.arange(128)[:, None])
    MASKS = np.concatenate(
        [NEGI, BIGM.astype(np.float32)], axis=1).astype(bf)      # (128, 256)

    in_maps = []
    scale = np.float32(HD ** -0.5)
    for c in range(NCORES):
        b, g = divmod(c, 4)
        qrows = slice(256 * g, 256 * g + 256)
        krows = slice(1024 + 256 * g, 1024 + 256 * g + 256)
        vrows = slice(2048 + 256 * g, 2048 + 256 * g + 256)

        XT = np.ascontiguousarray(x[b].T)                        # (1024, 2048)
        wq = (wqkv[qrows, :] * scale).T                          # (1024, 256)
        wk = wqkv[krows, :].T
        vpart = wqkv[vrows, :].T                                 # (1024, 256)
        WV = np.zeros((D, 260), dtype=np.float32)
        for h in range(HPC):
            WV[:, 65 * h:65 * h + 64] = vpart[:, 64 * h:64 * h + 64]
        # columns: [q01 | k01 | q23 | k23 | v] so the first-needed (m=0,2)
        # groups are a contiguous 256-col A-half
        WQKV = np.ascontiguousarray(np.concatenate(
            [wq[:, 0:128], wk[:, 0:128], wq[:, 128:256], wk[:, 128:256], WV],
            axis=1))                                             # (1024, 772)
        WOT = np.ascontiguousarray(wo[:, 256 * g:256 * g + 256].T)

        in_maps.append({
            "XT": XT.astype(np.float16),
            "WQKV": WQKV.astype(np.float16),
            "WOT": WOT.astype(np.float16),
            "CS": CSfull.astype(np.float16),
            "PERM": PERM.astype(np.float16),
            "MASKS": MASKS,
        })
    return in_maps


def kernel(x, wqkv, wo):
    if "nc" not in _cache:
        _cache["nc"] = build_nc()
    nc = _cache["nc"]
    in_maps = prep_inputs(x, wqkv, wo)
    res = bass_utils.run_bass_kernel_spmd(nc, in_maps, list(range(NCORES)))
    outs = [np.asarray(res.results[c]["OUT"], dtype=np.float32)
            for c in range(NCORES)]
    out0 = outs[0] + outs[1] + outs[2] + outs[3]
    out1 = outs[4] + outs[5] + outs[6] + outs[7]
    return np.stack([out0, out1]).astype(np.float32)


# revision 21
# speedup vs baseline: 1.2720x; 1.0280x over previous
"""Multi-head self-attention (RoPE, causal) Bass kernel for 8 TRN2 NeuronCores.

Problem: x (2, 2048, 1024) f32, wqkv (3072, 1024), wo (1024, 1024).
  qkv = x @ wqkv.T ; RoPE(q, k) ; causal softmax attention (16 heads, hd=64);
  out = y @ wo.T.

Sharding: batch (2-way) x head-group (4-way) tensor parallel = 8 cores.
Each core computes a full (2048, 1024) partial output for its batch from its
4 heads; host sums the 4 partials per batch (bf16 partials, f32 host sum).

v5 structure:
  - softmax exp on the scalar engine (~84us serial) is the bottleneck; the
    kernel is one flattened (wave, tile) pipeline that keeps the exp stream
    dense: scores+exp ("front") run LA tiles ahead of P@V ("back"), and
    qkv/wo groups fill the PE at deadline-chosen front positions.
  - startup: WQKV columns reordered [q01|k01|q23|k23|v] and loaded in an
    A (q01/k01) + B split so the first attention wave needs only ~1.5MB;
    xt loaded per 512-col chunk (chunks 0,1 via gpsimd SWDGE, 2,3 late on
    SP); cos/sin/perm on the scalar sequencer.
  - fp16 everywhere in the rope/scores path (fp32r runs the PE at half
    clock); bf16 -1e9 mask matmuls for the causal diagonal; fp16 P and V
    with an ones-column accumulating the softmax denominator.
  - first PV matmul of each wave uses start=True (psum bank zero) instead
    of DVE memsets; y_all -> yt via transpose DMA (PE transposes for the
    last chunk); chunk-0 PSUM->SBUF copies on the then-idle scalar engine.
"""
import sys

sys.path.insert(0, "/opt/trn_rl_repo")

import numpy as np

import concourse.bass as bass
import concourse.mybir as mybir
import concourse.tile as tile
from concourse import bacc, bass_utils
from concourse.masks import make_identity

B, L, D = 2, 2048, 1024
NH, HD = 16, 64
NCORES = 8
HPC = 4            # heads per core
LQB = 512          # Lq block per S^T unit
NLQ = L // LQB     # 4
NLT = L // 128     # 16
KT = D // 128      # 8 contraction tiles for projections

F32 = mybir.dt.float32
F16 = mybir.dt.float16
BF16 = mybir.dt.bfloat16

# WQKV column offset per rope group m (m=0,1: q pairs h01,h23; 2,3: k pairs)
MCOL = {0: 0, 2: 128, 1: 256, 3: 384}

_cache = {}


def build_nc(debug=False):
    nc = bacc.Bacc("TRN2", target_bir_lowering=False, debug=False)

    XT = nc.dram_tensor("XT", [D, L], F16, kind="ExternalInput")
    WQKV = nc.dram_tensor("WQKV", [D, 772], F16, kind="ExternalInput")
    WOT = nc.dram_tensor("WOT", [HPC * HD, D], F16, kind="ExternalInput")
    PERM = nc.dram_tensor("PERM", [128, 128], F16, kind="ExternalInput")
    CS = nc.dram_tensor("CS", [128, 2 * L], F16, kind="ExternalInput")
    MASKS = nc.dram_tensor("MASKS", [128, 256], BF16, kind="ExternalInput")
    OUT = nc.dram_tensor("OUT", [L, D], BF16, kind="ExternalOutput")

    with tile.TileContext(nc) as tc:
        with (
            tc.tile_pool(name="consts", bufs=1) as cpool,
            tc.tile_pool(name="weights", bufs=1) as wpool,
            tc.tile_pool(name="qkrot", bufs=1) as rotpool,
            tc.tile_pool(name="vsb", bufs=1) as vpool,
            tc.tile_pool(name="yall", bufs=1) as ypool,
            tc.tile_pool(name="ytr", bufs=1) as ytpool,
            tc.tile_pool(name="xt", bufs=1) as xpool,
            tc.tile_pool(name="tmps", bufs=4) as tpool,
            tc.tile_pool(name="raws", bufs=3) as rawpool,
            tc.tile_pool(name="pts", bufs=4) as ptpool,
            tc.tile_pool(name="outsb", bufs=3) as opool,
            tc.tile_pool(name="recs", bufs=4) as recpool,
            tc.tile_pool(name="psB", bufs=3, space="PSUM") as bigpool,
            tc.tile_pool(name="psY", bufs=1, space="PSUM") as psypool,
        ):
            # ---- input DMAs, split across idle sequencers ----------------
            wqkv_sb = [None] * KT
            xts = [[None] * KT for _ in range(NLQ)]
            cs_sb = [None] * NLQ

            perm_sb = cpool.tile([128, 128], F16, tag="perm")
            masks_sb = cpool.tile([128, 256], BF16, tag="masks")
            idn = cpool.tile([128, 128], F16, tag="idn")

            def load_xt(eng, j, k):
                t = xpool.tile([128, LQB], F16, tag=f"xt{j}_{k}",
                               name=f"xt{j}_{k}")
                eng.dma_start(t[:], XT[k * 128:(k + 1) * 128,
                                       j * LQB:(j + 1) * LQB])
                xts[j][k] = t

            # spread the wave-(0,*)-critical set (wqkv A-halves + xt chunk
            # 0) across three DMA queues: each queue sustains only ~90GB/s
            for k in range(KT):
                wqkv_sb[k] = wpool.tile([128, 772], F16, tag=f"wqkv{k}",
                                        name=f"wqkv{k}")
                nc.sync.dma_start(wqkv_sb[k][:, 0:256],
                                  WQKV[k * 128:(k + 1) * 128, 0:256])
            for k in range(4):
                load_xt(nc.gpsimd, 0, k)
            for k in range(4, KT):
                load_xt(nc.scalar, 0, k)
            cs_sb[0] = cpool.tile([128, 1024], F16, tag="cs0", name="cs0")
            nc.sync.dma_start(cs_sb[0][:], CS[:, 0:1024])
            nc.scalar.dma_start(perm_sb[:], PERM[:, :])
            # B-halves (q23|k23|v) next on SP; xt chunk 1 on gpsimd
            for k in range(KT):
                nc.sync.dma_start(wqkv_sb[k][:, 256:772],
                                  WQKV[k * 128:(k + 1) * 128, 256:772])
            for k in range(KT):
                load_xt(nc.gpsimd, 1, k)
            for j in range(1, NLQ):
                c = cpool.tile([128, 1024], F16, tag=f"cs{j}", name=f"cs{j}")
                nc.scalar.dma_start(c[:], CS[:, j * 1024:(j + 1) * 1024])
                cs_sb[j] = c
            nc.sync.dma_start(masks_sb[:], MASKS[:, :])
            wot_sb = []
            for c2 in range(2):
                w = wpool.tile([128, D], F16, tag=f"wo{c2}", name=f"wo{c2}")
                nc.sync.dma_start(w[:], WOT[c2 * 128:(c2 + 1) * 128, :])
                wot_sb.append(w)
            for j in range(2, NLQ):
                for k in range(KT):
                    load_xt(nc.sync, j, k)
            make_identity(nc, idn[:])

            # persistent activation storage
            qk_rot = [rotpool.tile([128, L], F16, tag=f"rot{m}",
                                   name=f"rot{m}")
                      for m in range(4)]
            v_sb = [vpool.tile([128, 260], F16, tag=f"v{t}", name=f"v{t}")
                    for t in range(NLT)]
            y_all = [ypool.tile([128, HPC * HD], F16, tag=f"y{i}", name=f"y{i}")
                     for i in range(NLT)]
            yt_sb = [ytpool.tile([128, L], F16, tag=f"yt{c2}", name=f"yt{c2}")
                     for c2 in range(2)]

            def ps_copy(j, out, in_):
                # chunk-0 copies ride the then-idle scalar engine
                if j == 0:
                    nc.scalar.copy(out, in_)
                else:
                    nc.vector.tensor_copy(out, in_)

            def qkv_m_parts(j, m):
                """Split a q/k projection+rope group into two ~1us PE
                micro-fillers so interleaved scores tiles aren't delayed."""
                xs = slice(j * LQB, (j + 1) * LQB)
                co = MCOL[m]
                st = {}

                def part_a():
                    big = bigpool.tile([128, 1024], F32, tag="big",
                                       name="big")
                    st["big"] = big
                    for k in range(4):
                        nc.tensor.matmul(
                            big[:, 0:512], wqkv_sb[k][:, co:co + 128],
                            xts[j][k][:], start=(k == 0), stop=False,
                        )

                def part_b():
                    big = st["big"]
                    ps = big[:, 0:512]
                    psw = big[:, 512:1024]
                    for k in range(4, KT):
                        nc.tensor.matmul(
                            ps, wqkv_sb[k][:, co:co + 128], xts[j][k][:],
                            start=False, stop=(k == KT - 1),
                        )
                    raw = rawpool.tile([128, LQB], F16, tag="raw")
                    ps_copy(j, raw[:], ps)
                    t1 = tpool.tile([128, LQB], F16, tag="t1")
                    nc.vector.tensor_mul(t1[:], raw[:], cs_sb[j][:, 0:512])
                    nc.tensor.matmul(psw, perm_sb[:], raw[:],
                                     start=True, stop=True)
                    t2 = tpool.tile([128, LQB], F16, tag="t2")
                    nc.vector.tensor_mul(t2[:], psw, cs_sb[j][:, 512:1024])
                    nc.vector.tensor_add(qk_rot[m][:, xs], t1[:], t2[:])

                return part_a, part_b

            def qkv_m_group(j, m):
                a, b = qkv_m_parts(j, m)
                a()
                b()

            def v_group(j, i2):
                ti = j * 4 + i2
                bigv = bigpool.tile([128, 1024], F32, tag="big", name="bigv")
                psv = bigv[:, 0:260]
                for k in range(KT):
                    nc.tensor.matmul(
                        psv, xts[j][k][:, i2 * 128:(i2 + 1) * 128],
                        wqkv_sb[k][:, 512:772],
                        start=(k == 0), stop=(k == KT - 1),
                    )
                ps_copy(j, v_sb[ti][:], psv)
                nc.vector.memset(v_sb[ti][:, 64:260:65], 1.0)

            # ---- attention: front (scores+exp) / back (P@V) pipeline ----
            y_ps = {}       # wave -> [tile, tile]
            pt_of = {}      # (wave, t) -> pt tile

            def front(w, t):
                jq, hp = w
                ks = slice(t * 128, (t + 1) * 128)
                diag = t >= 4 * jq
                off = max(0, t * 128 - jq * LQB)
                sp = bigpool.tile([128, 1024], F32, tag="big", name="sp")
                for h in range(2):
                    hs = slice(64 * h, 64 * h + 64)
                    nc.tensor.matmul(
                        sp[:, 512 * h + off:512 * h + 512],
                        qk_rot[2 + hp][hs, ks],
                        qk_rot[hp][hs, jq * LQB + off:(jq + 1) * LQB],
                        start=True, stop=not diag,
                    )
                if diag:
                    for h in range(2):
                        nc.tensor.matmul(
                            sp[:, 512 * h + off:512 * h + off + 128],
                            masks_sb[:, 0:128],
                            masks_sb[:, 128:256],
                            start=False, stop=True,
                        )
                pt = ptpool.tile([128, 1024], F16, tag="pt")
                nc.scalar.activation(
                    pt[:, off:1024], sp[:, off:1024],
                    mybir.ActivationFunctionType.Exp
                )
                pt_of[(w, t)] = pt

            def back(w, t):
                jq, hp = w
                nt = 4 * jq + 4
                off = max(0, t * 128 - jq * LQB)
                r = off // 128
                if t == 0:
                    y_ps[w] = [psypool.tile([128, 260], F32, tag=f"yps{h}",
                                            name=f"yps{h}", bufs=1)
                               for h in range(2)]
                pt = pt_of.pop((w, t))
                for h in range(2):
                    H = 2 * hp + h
                    for js in range(r, 4):
                        nc.tensor.matmul(
                            y_ps[w][h][:, 65 * js:65 * js + 65],
                            pt[:, 512 * h + 128 * js:512 * h + 128 * js + 128],
                            v_sb[t][:, 65 * H:65 * H + 65],
                            # first matmul of the wave zeroes the whole
                            # psum bank (the tile owns it); PE stream order
                            # guarantees it runs before the others
                            start=(t == 0 and js == 0), stop=(t == nt - 1),
                            skip_group_check=True,
                        )

            def wave_end(w):
                jq, hp = w
                fused_tail = (w == (NLQ - 1, 1))
                recs = []
                for h in range(2):
                    rec = recpool.tile([128, 4], F32, tag="rec")
                    nc.vector.reciprocal(rec[:], y_ps[w][h][:, 64:260:65])
                    recs.append(rec)
                for js in range(4):
                    i = 4 * jq + js
                    for h in range(2):
                        H = 2 * hp + h
                        nc.vector.tensor_scalar_mul(
                            y_all[i][:, HD * H:HD * H + HD],
                            y_ps[w][h][:, 65 * js:65 * js + 64],
                            recs[h][:, js:js + 1],
                        )
                    if fused_tail:
                        # last chunk: transpose + project + store this
                        # q-tile immediately (shortens the serial tail)
                        transpose_one(i, pe=True)
                        wo_tile(i)
                del y_ps[w]

            def transpose_one(i, pe=False):
                if not pe:
                    for c2 in range(2):
                        nc.sync.dma_start(
                            yt_sb[c2][:, 128 * i:128 * i + 128],
                            y_all[i][:, 128 * c2:128 * c2 + 128],
                            transpose=True,
                        )
                else:
                    bigt = bigpool.tile([128, 1024], F32, tag="big",
                                        name="bigt")
                    tp16 = bigt[:].bitcast(F16)
                    for c2 in range(2):
                        nc.tensor.transpose(
                            tp16[:, 128 * c2:128 * c2 + 128],
                            y_all[i][:, 128 * c2:128 * c2 + 128],
                            idn[:],
                        )
                        nc.vector.tensor_copy(
                            yt_sb[c2][:, 128 * i:128 * i + 128],
                            tp16[:, 128 * c2:128 * c2 + 128],
                        )

            def transpose_tiles(jq):
                for i in range(4 * jq, 4 * jq + 4):
                    transpose_one(i)

            def wo_tile(i):
                po = bigpool.tile([128, 1024], F32, tag="big", name="po")
                for half in range(2):
                    for c2 in range(2):
                        nc.tensor.matmul(
                            po[:, 512 * half:512 * half + 512],
                            yt_sb[c2][:, 128 * i:128 * i + 128],
                            wot_sb[c2][:, 512 * half:512 * half + 512],
                            start=(c2 == 0), stop=(c2 == 1),
                        )
                ob = opool.tile([128, 1024], BF16, tag="ob")
                nc.vector.tensor_copy(ob[:], po[:])
                nc.sync.dma_start(OUT[128 * i:128 * i + 128, :], ob[:])

            # ---- main schedule: flattened lookahead pipeline ------------
            LA = 2
            waves = [(jq, hp) for jq in range(NLQ) for hp in range(2)]
            nts = {w: 4 * w[0] + 4 for w in waves}
            seq = [(w, t) for w in waves for t in range(nts[w])]
            # window jq -> offset of its first front position
            woff = {}
            p = 0
            for w in waves:
                if w[1] == 0:
                    woff[w[0]] = p
                p += nts[w]

            def V(j, i2):
                return lambda: v_group(j, i2)

            def WO(i):
                return lambda: wo_tile(i)

            # explicit deadline-aware micro-filler positions
            # (window-relative; each item <= ~1us of PE time)
            def win_filler_table():
                MP = {(j, m): qkv_m_parts(j, m)
                      for j in range(NLQ) for m in range(4)
                      if (j, m) not in ((0, 0), (0, 2))}

                def MA(j, m):
                    return MP[(j, m)][0]

                def MB(j, m):
                    return MP[(j, m)][1]

                return {
                    0: [(0, V(0, 0)), (0, MA(0, 1)), (1, MB(0, 1)),
                        (2, MA(0, 3)), (2, V(0, 1)), (3, MB(0, 3)),
                        (4, V(0, 2)), (5, V(0, 3)),
                        (6, MA(1, 0)), (6, MB(1, 0)),
                        (7, MA(1, 2)), (7, MB(1, 2))],
                    1: [(0, MA(1, 1)), (1, MB(1, 1)), (2, V(1, 0)),
                        (3, MA(1, 3)), (4, MB(1, 3)), (5, V(1, 1)),
                        (6, V(1, 2)), (7, V(1, 3)),
                        (9, MA(2, 0)), (10, MB(2, 0)),
                        (12, MA(2, 2)), (13, MB(2, 2))],
                    2: [(0, MA(2, 1)), (1, MB(2, 1)), (2, V(2, 0)),
                        (3, MA(2, 3)), (4, MB(2, 3)), (5, V(2, 1)),
                        (7, V(2, 2)), (9, V(2, 3)),
                        (11, MA(3, 0)), (12, MB(3, 0)),
                        (14, MA(3, 2)), (15, MB(3, 2)),
                        (17, WO(0)), (19, WO(1)), (21, WO(2)), (23, WO(3))],
                    3: [(0, MA(3, 1)), (1, MB(3, 1)), (2, V(3, 0)),
                        (3, MA(3, 3)), (4, MB(3, 3)), (5, V(3, 1)),
                        (7, V(3, 2)), (9, V(3, 3)),
                        (12, WO(4)), (14, WO(5)), (16, WO(6)), (18, WO(7)),
                        (20, WO(8)), (22, WO(9)), (24, WO(10)), (26, WO(11))],
                }

            fpos = {}
            for jq, fl in win_filler_table().items():
                for rel, fn in fl:
                    fpos.setdefault(woff[jq] + rel, []).append(fn)

            def emit_back(q):
                wq, tq = seq[q]
                back(wq, tq)
                if tq == nts[wq] - 1:
                    wave_end(wq)
                    if wq[1] == 1 and wq[0] < NLQ - 1:
                        transpose_tiles(wq[0])

            qkv_m_group(0, 0)
            qkv_m_group(0, 2)
            for p, (w, t) in enumerate(seq):
                for fn in fpos.get(p, ()):
                    fn()
                front(w, t)
                if p - LA >= 0:
                    emit_back(p - LA)
            for q in range(len(seq) - LA, len(seq)):
                emit_back(q)

    nc.finalize()
    return nc


def prep_inputs(x, wqkv, wo):
    """Build the 8 per-core input dicts from the full-problem inputs."""
    import ml_dtypes

    x = np.asarray(x, dtype=np.float32)
    wqkv = np.asarray(wqkv, dtype=np.float32)
    wo = np.asarray(wo, dtype=np.float32)

    # rope tables; CS: per chunk j a [cos_j (512) | sin_j (512)] block
    inv_freq = 1.0 / (10000.0 ** (np.arange(0, HD, 2, dtype=np.float32) / HD))
    t = np.arange(L, dtype=np.float32)
    freqs = np.outer(t, inv_freq)                  # (L, 32)
    cos32 = np.cos(freqs).T.astype(np.float32)     # (32, L)
    sin32 = np.sin(freqs).T.astype(np.float32)
    COS = np.tile(cos32, (4, 1))                                 # (128, L)
    SIN = np.concatenate([-sin32, sin32, -sin32, sin32], axis=0)
    CSfull = np.zeros((128, 2 * L), dtype=np.float32)
    for j in range(NLQ):
        CSfull[:, 1024 * j:1024 * j + 512] = COS[:, 512 * j:512 * j + 512]
        CSfull[:, 1024 * j + 512:1024 * j + 1024] = \
            SIN[:, 512 * j:512 * j + 512]

    # 32-block swap permutation (within each head's 64 rows)
    PERM = np.zeros((128, 128), dtype=np.float32)
    for blk in range(2):
        o = 64 * blk
        PERM[o:o + 32, o + 32:o + 64] = np.eye(32)
        PERM[o + 32:o + 64, o:o + 32] = np.eye(32)

    bf = ml_dtypes.bfloat16
    NEGI = (-1e9 * np.eye(128)).astype(np.float32)
    BIGM = (np.arange(128)[None, :] < np.arange(128)[:, None])
    MASKS = np.concatenate(
        [NEGI, BIGM.astype(np.float32)], axis=1).astype(bf)      # (128, 256)

    in_maps = []
    scale = np.float32(HD ** -0.5)
    for c in range(NCORES):
        b, g = divmod(c, 4)
        qrows = slice(256 * g, 256 * g + 256)
        krows = slice(1024 + 256 * g, 1024 + 256 * g + 256)
        vrows = slice(2048 + 256 * g, 2048 + 256 * g + 256)

        XT = np.ascontiguousarray(x[b].T)                        # (1024, 2048)
        wq = (wqkv[qrows, :] * scale).T                          # (1024, 256)
        wk = wqkv[krows, :].T
        vpart = wqkv[vrows, :].T                                 # (1024, 256)
        WV = np.zeros((D, 260), dtype=np.float32)
        for h in range(HPC):
            WV[:, 65 * h:65 * h + 64] = vpart[:, 64 * h:64 * h + 64]
        # columns: [q01 | k01 | q23 | k23 | v] so the first-needed (m=0,2)
        # groups are a contiguous 256-col A-half
        WQKV = np.ascontiguousarray(np.concatenate(
            [wq[:, 0:128], wk[:, 0:128], wq[:, 128:256], wk[:, 128:256], WV],
            axis=1))                                             # (1024, 772)
        WOT = np.ascontiguousarray(wo[:, 256 * g:256 * g + 256].T)

        in_maps.append({
            "XT": XT.astype(np.float16),
            "WQKV": WQKV.astype(np.float16),
            "WOT": WOT.astype(np.float16),
            "CS": CSfull.astype(np.float16),
            "PERM": PERM.astype(np.float16),
            "MASKS": MASKS,
        })
    return in_maps


def kernel(x, wqkv, wo):
    if "nc" not in _cache:
        _cache["nc"] = build_nc()
    nc = _cache["nc"]
    in_maps = prep_inputs(x, wqkv, wo)
    res = bass_utils.run_bass_kernel_spmd(nc, in_maps, list(range(NCORES)))
    outs = [np.asarray(res.results[c]["OUT"], dtype=np.float32)
            for c in range(NCORES)]
    out0 = outs[0] + outs[1] + outs[2] + outs[3]
    out1 = outs[4] + outs[5] + outs[6] + outs[7]
    return np.stack([out0, out1]).astype(np.float32)


# revision 26
# speedup vs baseline: 1.2919x; 1.0157x over previous
"""Multi-head self-attention (RoPE, causal) Bass kernel for 8 TRN2 NeuronCores.

Problem: x (2, 2048, 1024) f32, wqkv (3072, 1024), wo (1024, 1024).
  qkv = x @ wqkv.T ; RoPE(q, k) ; causal softmax attention (16 heads, hd=64);
  out = y @ wo.T.

Sharding: batch (2-way) x head-group (4-way) tensor parallel = 8 cores.
Each core computes a full (2048, 1024) partial output for its batch from its
4 heads; host sums the 4 partials per batch (bf16 partials, f32 host sum).

v5 structure:
  - softmax exp on the scalar engine (~84us serial) is the bottleneck; the
    kernel is one flattened (wave, tile) pipeline that keeps the exp stream
    dense: scores+exp ("front") run LA tiles ahead of P@V ("back"), and
    qkv/wo groups fill the PE at deadline-chosen front positions.
  - startup: WQKV columns reordered [q01|k01|q23|k23|v] and loaded in an
    A (q01/k01) + B split so the first attention wave needs only ~1.5MB;
    xt loaded per 512-col chunk (chunks 0,1 via gpsimd SWDGE, 2,3 late on
    SP); cos/sin/perm on the scalar sequencer.
  - fp16 everywhere in the rope/scores path (fp32r runs the PE at half
    clock); bf16 -1e9 mask matmuls for the causal diagonal; fp16 P and V
    with an ones-column accumulating the softmax denominator.
  - first PV matmul of each wave uses start=True (psum bank zero) instead
    of DVE memsets; y_all -> yt via transpose DMA (PE transposes for the
    last chunk); chunk-0 PSUM->SBUF copies on the then-idle scalar engine.
"""
import sys

sys.path.insert(0, "/opt/trn_rl_repo")

import numpy as np

import concourse.bass as bass
import concourse.mybir as mybir
import concourse.tile as tile
from concourse import bacc, bass_utils
from concourse.masks import make_identity

B, L, D = 2, 2048, 1024
NH, HD = 16, 64
NCORES = 8
HPC = 4            # heads per core
LQB = 512          # Lq block per S^T unit
NLQ = L // LQB     # 4
NLT = L // 128     # 16
KT = D // 128      # 8 contraction tiles for projections

F32 = mybir.dt.float32
F16 = mybir.dt.float16
BF16 = mybir.dt.bfloat16

# WQKV column offset per rope group m (m=0,1: q pairs h01,h23; 2,3: k pairs)
MCOL = {0: 0, 2: 128, 1: 256, 3: 384}

_cache = {}


def build_nc(debug=False):
    nc = bacc.Bacc("TRN2", target_bir_lowering=False, debug=False)

    XT = nc.dram_tensor("XT", [D, L], F16, kind="ExternalInput")
    WQKV = nc.dram_tensor("WQKV", [D, 772], F16, kind="ExternalInput")
    WOT = nc.dram_tensor("WOT", [HPC * HD, D], F16, kind="ExternalInput")
    PERM = nc.dram_tensor("PERM", [128, 128], F16, kind="ExternalInput")
    CS = nc.dram_tensor("CS", [128, 2 * L], F16, kind="ExternalInput")
    MASKS = nc.dram_tensor("MASKS", [128, 256], BF16, kind="ExternalInput")
    OUT = nc.dram_tensor("OUT", [L, D], BF16, kind="ExternalOutput")

    with tile.TileContext(nc) as tc:
        with (
            tc.tile_pool(name="consts", bufs=1) as cpool,
            tc.tile_pool(name="weights", bufs=1) as wpool,
            tc.tile_pool(name="qkrot", bufs=1) as rotpool,
            tc.tile_pool(name="vsb", bufs=1) as vpool,
            tc.tile_pool(name="yall", bufs=1) as ypool,
            tc.tile_pool(name="ytr", bufs=1) as ytpool,
            tc.tile_pool(name="xt", bufs=1) as xpool,
            tc.tile_pool(name="tmps", bufs=4) as tpool,
            tc.tile_pool(name="raws", bufs=3) as rawpool,
            tc.tile_pool(name="pts", bufs=4) as ptpool,
            tc.tile_pool(name="outsb", bufs=3) as opool,
            tc.tile_pool(name="recs", bufs=4) as recpool,
            tc.tile_pool(name="psB", bufs=3, space="PSUM") as bigpool,
            tc.tile_pool(name="psY", bufs=1, space="PSUM") as psypool,
        ):
            # ---- input DMAs, split across idle sequencers ----------------
            wqkv_sb = [None] * KT
            xts = [[None] * KT for _ in range(NLQ)]
            cs_sb = [None] * NLQ

            perm_sb = cpool.tile([128, 128], F16, tag="perm")
            masks_sb = cpool.tile([128, 256], BF16, tag="masks")
            idn = cpool.tile([128, 128], F16, tag="idn")

            def load_xt(eng, j, k):
                t = xpool.tile([128, LQB], F16, tag=f"xt{j}_{k}",
                               name=f"xt{j}_{k}")
                eng.dma_start(t[:], XT[k * 128:(k + 1) * 128,
                                       j * LQB:(j + 1) * LQB])
                xts[j][k] = t

            # spread the wave-(0,*)-critical set (wqkv A-halves + xt chunk
            # 0) across three DMA queues: each queue sustains only ~90GB/s
            for k in range(KT):
                wqkv_sb[k] = wpool.tile([128, 772], F16, tag=f"wqkv{k}",
                                        name=f"wqkv{k}")
                nc.sync.dma_start(wqkv_sb[k][:, 0:256],
                                  WQKV[k * 128:(k + 1) * 128, 0:256])
            for k in range(4):
                load_xt(nc.gpsimd, 0, k)
            for k in range(4, KT):
                load_xt(nc.scalar, 0, k)
            cs_sb[0] = cpool.tile([128, 1024], F16, tag="cs0", name="cs0")
            nc.sync.dma_start(cs_sb[0][:], CS[:, 0:1024])
            nc.scalar.dma_start(perm_sb[:], PERM[:, :])
            # B-halves (q23|k23|v) split across the SP and scalar queues;
            # xt chunk 1 on gpsimd
            for k in range(4):
                nc.sync.dma_start(wqkv_sb[k][:, 256:772],
                                  WQKV[k * 128:(k + 1) * 128, 256:772])
            for k in range(4, KT):
                nc.scalar.dma_start(wqkv_sb[k][:, 256:772],
                                    WQKV[k * 128:(k + 1) * 128, 256:772])
            for k in range(KT):
                load_xt(nc.gpsimd, 1, k)
            for j in range(1, NLQ):
                c = cpool.tile([128, 1024], F16, tag=f"cs{j}", name=f"cs{j}")
                nc.scalar.dma_start(c[:], CS[:, j * 1024:(j + 1) * 1024])
                cs_sb[j] = c
            nc.sync.dma_start(masks_sb[:], MASKS[:, :])
            wot_sb = []
            for c2 in range(2):
                w = wpool.tile([128, D], F16, tag=f"wo{c2}", name=f"wo{c2}")
                nc.sync.dma_start(w[:], WOT[c2 * 128:(c2 + 1) * 128, :])
                wot_sb.append(w)
            for j in range(2, NLQ):
                for k in range(KT):
                    load_xt(nc.sync, j, k)
            make_identity(nc, idn[:])

            # persistent activation storage
            qk_rot = [rotpool.tile([128, L], F16, tag=f"rot{m}",
                                   name=f"rot{m}")
                      for m in range(4)]
            v_sb = [vpool.tile([128, 260], F16, tag=f"v{t}", name=f"v{t}")
                    for t in range(NLT)]
            y_all = [ypool.tile([128, HPC * HD], F16, tag=f"y{i}", name=f"y{i}")
                     for i in range(NLT)]
            yt_sb = [ytpool.tile([128, L], F16, tag=f"yt{c2}", name=f"yt{c2}")
                     for c2 in range(2)]

            def ps_copy(j, out, in_):
                # chunk-0 copies ride the then-idle scalar engine
                if j == 0:
                    nc.scalar.copy(out, in_)
                else:
                    nc.vector.tensor_copy(out, in_)

            def qkv_m_parts(j, m):
                """Split a q/k projection+rope group into two ~1us PE
                micro-fillers so interleaved scores tiles aren't delayed."""
                xs = slice(j * LQB, (j + 1) * LQB)
                co = MCOL[m]
                st = {}

                def part_a():
                    big = bigpool.tile([128, 1024], F32, tag="big",
                                       name="big")
                    st["big"] = big
                    for k in range(4):
                        nc.tensor.matmul(
                            big[:, 0:512], wqkv_sb[k][:, co:co + 128],
                            xts[j][k][:], start=(k == 0), stop=False,
                        )

                def part_b():
                    big = st["big"]
                    ps = big[:, 0:512]
                    psw = big[:, 512:1024]
                    for k in range(4, KT):
                        nc.tensor.matmul(
                            ps, wqkv_sb[k][:, co:co + 128], xts[j][k][:],
                            start=False, stop=(k == KT - 1),
                        )
                    raw = rawpool.tile([128, LQB], F16, tag="raw")
                    ps_copy(j, raw[:], ps)
                    t1 = tpool.tile([128, LQB], F16, tag="t1")
                    nc.vector.tensor_mul(t1[:], raw[:], cs_sb[j][:, 0:512])
                    nc.tensor.matmul(psw, perm_sb[:], raw[:],
                                     start=True, stop=True)
                    t2 = tpool.tile([128, LQB], F16, tag="t2")
                    nc.vector.tensor_mul(t2[:], psw, cs_sb[j][:, 512:1024])
                    nc.vector.tensor_add(qk_rot[m][:, xs], t1[:], t2[:])

                return part_a, part_b

            def qkv_m_group(j, m):
                a, b = qkv_m_parts(j, m)
                a()
                b()

            def rope_finish(j, m, big):
                xs = slice(j * LQB, (j + 1) * LQB)
                ps = big[:, 0:512]
                psw = big[:, 512:1024]
                raw = rawpool.tile([128, LQB], F16, tag="raw")
                ps_copy(j, raw[:], ps)
                t1 = tpool.tile([128, LQB], F16, tag="t1")
                nc.vector.tensor_mul(t1[:], raw[:], cs_sb[j][:, 0:512])
                nc.tensor.matmul(psw, perm_sb[:], raw[:],
                                 start=True, stop=True)
                t2 = tpool.tile([128, LQB], F16, tag="t2")
                nc.vector.tensor_mul(t2[:], psw, cs_sb[j][:, 512:1024])
                nc.vector.tensor_add(qk_rot[m][:, xs], t1[:], t2[:])

            def qkv0_head(ms=(0, 2)):
                """Chunk-0 q01/k01 groups with the k-loop interleaved and
                ordered by DMA arrival (two xt queues land k0-3 and k4-7 in
                parallel) so the PE consumes tiles as they land."""
                bigs = {}
                for m in ms:
                    bigs[m] = bigpool.tile([128, 1024], F32, tag="big",
                                           name="big")
                order = [0, 4, 1, 5, 2, 6, 3, 7]
                for idx, kk in enumerate(order):
                    for m in ms:
                        nc.tensor.matmul(
                            bigs[m][:, 0:512],
                            wqkv_sb[kk][:, MCOL[m]:MCOL[m] + 128],
                            xts[0][kk][:],
                            start=(idx == 0), stop=(idx == len(order) - 1),
                            skip_group_check=True,
                        )
                for m in ms:
                    rope_finish(0, m, bigs[m])

            def v_group(j, i2):
                ti = j * 4 + i2
                bigv = bigpool.tile([128, 1024], F32, tag="big", name="bigv")
                psv = bigv[:, 0:260]
                for k in range(KT):
                    nc.tensor.matmul(
                        psv, xts[j][k][:, i2 * 128:(i2 + 1) * 128],
                        wqkv_sb[k][:, 512:772],
                        start=(k == 0), stop=(k == KT - 1),
                    )
                ps_copy(j, v_sb[ti][:], psv)
                nc.vector.memset(v_sb[ti][:, 64:260:65], 1.0)

            # ---- attention: front (scores+exp) / back (P@V) pipeline ----
            y_ps = {}       # wave -> [tile, tile]
            pt_of = {}      # (wave, t) -> pt tile

            def front(w, t):
                jq, hp = w
                ks = slice(t * 128, (t + 1) * 128)
                diag = t >= 4 * jq
                off = max(0, t * 128 - jq * LQB)
                sp = bigpool.tile([128, 1024], F32, tag="big", name="sp")
                for h in range(2):
                    hs = slice(64 * h, 64 * h + 64)
                    nc.tensor.matmul(
                        sp[:, 512 * h + off:512 * h + 512],
                        qk_rot[2 + hp][hs, ks],
                        qk_rot[hp][hs, jq * LQB + off:(jq + 1) * LQB],
                        start=True, stop=not diag,
                    )
                if diag:
                    for h in range(2):
                        nc.tensor.matmul(
                            sp[:, 512 * h + off:512 * h + off + 128],
                            masks_sb[:, 0:128],
                            masks_sb[:, 128:256],
                            start=False, stop=True,
                        )
                pt = ptpool.tile([128, 1024], F16, tag="pt")
                nc.scalar.activation(
                    pt[:, off:1024], sp[:, off:1024],
                    mybir.ActivationFunctionType.Exp
                )
                pt_of[(w, t)] = pt

            def back(w, t):
                jq, hp = w
                nt = 4 * jq + 4
                off = max(0, t * 128 - jq * LQB)
                r = off // 128
                if t == 0:
                    y_ps[w] = [psypool.tile([128, 260], F32, tag=f"yps{h}",
                                            name=f"yps{h}", bufs=1)
                               for h in range(2)]
                pt = pt_of.pop((w, t))
                for h in range(2):
                    H = 2 * hp + h
                    for js in range(r, 4):
                        nc.tensor.matmul(
                            y_ps[w][h][:, 65 * js:65 * js + 65],
                            pt[:, 512 * h + 128 * js:512 * h + 128 * js + 128],
                            v_sb[t][:, 65 * H:65 * H + 65],
                            # first matmul of the wave zeroes the whole
                            # psum bank (the tile owns it); PE stream order
                            # guarantees it runs before the others
                            start=(t == 0 and js == 0), stop=(t == nt - 1),
                            skip_group_check=True,
                        )

            def wave_end(w):
                jq, hp = w
                fused_tail = (w == (NLQ - 1, 1))
                recs = []
                for h in range(2):
                    rec = recpool.tile([128, 4], F32, tag="rec")
                    nc.vector.reciprocal(rec[:], y_ps[w][h][:, 64:260:65])
                    recs.append(rec)
                for js in range(4):
                    i = 4 * jq + js
                    for h in range(2):
                        H = 2 * hp + h
                        nc.vector.tensor_scalar_mul(
                            y_all[i][:, HD * H:HD * H + HD],
                            y_ps[w][h][:, 65 * js:65 * js + 64],
                            recs[h][:, js:js + 1],
                        )
                    if fused_tail:
                        # last chunk: transpose + project + store this
                        # q-tile immediately (shortens the serial tail);
                        # ob copies on the now-idle scalar engine
                        transpose_one(i, pe=True)
                        wo_tile(i, ob_eng="scalar")
                del y_ps[w]

            def transpose_one(i, pe=False):
                if not pe:
                    for c2 in range(2):
                        nc.sync.dma_start(
                            yt_sb[c2][:, 128 * i:128 * i + 128],
                            y_all[i][:, 128 * c2:128 * c2 + 128],
                            transpose=True,
                        )
                else:
                    bigt = bigpool.tile([128, 1024], F32, tag="big",
                                        name="bigt")
                    tp16 = bigt[:].bitcast(F16)
                    for c2 in range(2):
                        nc.tensor.transpose(
                            tp16[:, 128 * c2:128 * c2 + 128],
                            y_all[i][:, 128 * c2:128 * c2 + 128],
                            idn[:],
                        )
                        nc.vector.tensor_copy(
                            yt_sb[c2][:, 128 * i:128 * i + 128],
                            tp16[:, 128 * c2:128 * c2 + 128],
                        )

            def transpose_tiles(jq):
                for i in range(4 * jq, 4 * jq + 4):
                    transpose_one(i)

            def wo_tile(i, ob_eng=None):
                po = bigpool.tile([128, 1024], F32, tag="big", name="po")
                for half in range(2):
                    for c2 in range(2):
                        nc.tensor.matmul(
                            po[:, 512 * half:512 * half + 512],
                            yt_sb[c2][:, 128 * i:128 * i + 128],
                            wot_sb[c2][:, 512 * half:512 * half + 512],
                            start=(c2 == 0), stop=(c2 == 1),
                        )
                ob = opool.tile([128, 1024], BF16, tag="ob")
                if ob_eng == "scalar":
                    nc.scalar.copy(ob[:], po[:])
                else:
                    nc.vector.tensor_copy(ob[:], po[:])
                nc.sync.dma_start(OUT[128 * i:128 * i + 128, :], ob[:])

            # ---- main schedule: flattened lookahead pipeline ------------
            LA = 2
            waves = [(jq, hp) for jq in range(NLQ) for hp in range(2)]
            nts = {w: 4 * w[0] + 4 for w in waves}
            seq = [(w, t) for w in waves for t in range(nts[w])]
            # window jq -> offset of its first front position
            woff = {}
            p = 0
            for w in waves:
                if w[1] == 0:
                    woff[w[0]] = p
                p += nts[w]

            def V(j, i2):
                return lambda: v_group(j, i2)

            def WO(i):
                return lambda: wo_tile(i)

            # explicit deadline-aware micro-filler positions
            # (window-relative; each item <= ~1us of PE time)
            def win_filler_table():
                MP = {(j, m): qkv_m_parts(j, m)
                      for j in range(NLQ) for m in range(4)
                      if (j, m) not in ((0, 0), (0, 2))}

                def MA(j, m):
                    return MP[(j, m)][0]

                def MB(j, m):
                    return MP[(j, m)][1]

                return {
                    0: [(0, V(0, 0)), (0, MA(0, 1)), (1, MB(0, 1)),
                        (2, MA(0, 3)), (2, V(0, 1)), (3, MB(0, 3)),
                        (4, V(0, 2)), (5, V(0, 3)),
                        (6, MA(1, 0)), (6, MB(1, 0)),
                        (7, MA(1, 2)), (7, MB(1, 2))],
                    1: [(0, MA(1, 1)), (1, MB(1, 1)), (2, V(1, 0)),
                        (3, MA(1, 3)), (4, MB(1, 3)), (5, V(1, 1)),
                        (6, V(1, 2)), (7, V(1, 3)),
                        (9, MA(2, 0)), (10, MB(2, 0)),
                        (12, MA(2, 2)), (13, MB(2, 2))],
                    2: [(0, MA(2, 1)), (1, MB(2, 1)), (2, V(2, 0)),
                        (3, MA(2, 3)), (4, MB(2, 3)), (5, V(2, 1)),
                        (7, V(2, 2)), (9, V(2, 3)),
                        (11, MA(3, 0)), (12, MB(3, 0)),
                        (14, MA(3, 2)), (15, MB(3, 2)),
                        (17, WO(0)), (19, WO(1)), (21, WO(2)), (23, WO(3))],
                    3: [(0, MA(3, 1)), (1, MB(3, 1)), (2, V(3, 0)),
                        (3, MA(3, 3)), (4, MB(3, 3)), (5, V(3, 1)),
                        (7, V(3, 2)), (9, V(3, 3)),
                        (12, WO(4)), (14, WO(5)), (16, WO(6)), (18, WO(7)),
                        (20, WO(8)), (22, WO(9)), (24, WO(10)), (26, WO(11))],
                }

            fpos = {}
            for jq, fl in win_filler_table().items():
                for rel, fn in fl:
                    fpos.setdefault(woff[jq] + rel, []).append(fn)

            def emit_back(q):
                wq, tq = seq[q]
                back(wq, tq)
                if tq == nts[wq] - 1:
                    wave_end(wq)
                    if wq[1] == 1 and wq[0] < NLQ - 1:
                        transpose_tiles(wq[0])

            qkv0_head()
            for p, (w, t) in enumerate(seq):
                for fn in fpos.get(p, ()):
                    fn()
                front(w, t)
                if p - LA >= 0:
                    emit_back(p - LA)
            for q in range(len(seq) - LA, len(seq)):
                emit_back(q)

    nc.finalize()
    return nc


def prep_inputs(x, wqkv, wo):
    """Build the 8 per-core input dicts from the full-problem inputs."""
    import ml_dtypes

    x = np.asarray(x, dtype=np.float32)
    wqkv = np.asarray(wqkv, dtype=np.float32)
    wo = np.asarray(wo, dtype=np.float32)

    # rope tables; CS: per chunk j a [cos_j (512) | sin_j (512)] block
    inv_freq = 1.0 / (10000.0 ** (np.arange(0, HD, 2, dtype=np.float32) / HD))
    t = np.arange(L, dtype=np.float32)
    freqs = np.outer(t, inv_freq)                  # (L, 32)
    cos32 = np.cos(freqs).T.astype(np.float32)     # (32, L)
    sin32 = np.sin(freqs).T.astype(np.float32)
    COS = np.tile(cos32, (4, 1))                                 # (128, L)
    SIN = np.concatenate([-sin32, sin32, -sin32, sin32], axis=0)
    CSfull = np.zeros((128, 2 * L), dtype=np.float32)
    for j in range(NLQ):
        CSfull[:, 1024 * j:1024 * j + 512] = COS[:, 512 * j:512 * j + 512]
        CSfull[:, 1024 * j + 512:1024 * j + 1024] = \
            SIN[:, 512 * j:512 * j + 512]

    # 32-block swap permutation (within each head's 64 rows)
    PERM = np.zeros((128, 128), dtype=np.float32)
    for blk in range(2):
        o = 64 * blk
        PERM[o:o + 32, o + 32:o + 64] = np.eye(32)
        PERM[o + 32:o + 64, o:o + 32] = np.eye(32)

    bf = ml_dtypes.bfloat16
    NEGI = (-1e9 * np.eye(128)).astype(np.float32)
    BIGM = (np.arange(128)[None, :] < np.arange(128)[:, None])
    MASKS = np.concatenate(
        [NEGI, BIGM.astype(np.float32)], axis=1).astype(bf)      # (128, 256)

    in_maps = []
    scale = np.float32(HD ** -0.5)
    for c in range(NCORES):
        b, g = divmod(c, 4)
        qrows = slice(256 * g, 256 * g + 256)
        krows = slice(1024 + 256 * g, 1024 + 256 * g + 256)
        vrows = slice(2048 + 256 * g, 2048 + 256 * g + 256)

        XT = np.ascontiguousarray(x[b].T)                        # (1024, 2048)
        wq = (wqkv[qrows, :] * scale).T                          # (1024, 256)
        wk = wqkv[krows, :].T
        vpart = wqkv[vrows, :].T                                 # (1024, 256)
        WV = np.zeros((D, 260), dtype=np.float32)
        for h in range(HPC):
            WV[:, 65 * h:65 * h + 64] = vpart[:, 64 * h:64 * h + 64]
        # columns: [q01 | k01 | q23 | k23 | v] so the first-needed (m=0,2)
        # groups are a contiguous 256-col A-half
        WQKV = np.ascontiguousarray(np.concatenate(
            [wq[:, 0:128], wk[:, 0:128], wq[:, 128:256], wk[:, 128:256], WV],
            axis=1))                                             # (1024, 772)
        WOT = np.ascontiguousarray(wo[:, 256 * g:256 * g + 256].T)

        in_maps.append({
            "XT": XT.astype(np.float16),
            "WQKV": WQKV.astype(np.float16),
            "WOT": WOT.astype(np.float16),
            "CS": CSfull.astype(np.float16),
            "PERM": PERM.astype(np.float16),
            "MASKS": MASKS,
        })
    return in_maps


def kernel(x, wqkv, wo):
    if "nc" not in _cache:
        _cache["nc"] = build_nc()
    nc = _cache["nc"]
    in_maps = prep_inputs(x, wqkv, wo)
    res = bass_utils.run_bass_kernel_spmd(nc, in_maps, list(range(NCORES)))
    outs = [np.asarray(res.results[c]["OUT"], dtype=np.float32)
            for c in range(NCORES)]
    out0 = outs[0] + outs[1] + outs[2] + outs[3]
    out1 = outs[4] + outs[5] + outs[6] + outs[7]
    return np.stack([out0, out1]).astype(np.float32)
